# revision 1
# baseline (speedup 1.0000x reference)
"""Fused multi-head attention block (qkv proj + RMSNorm(q,k) + softmax(QK^T)V
+ out proj), tensor-parallel over 8 TRN2 NeuronCores (2 heads per core).

Layout strategy (per core):
  - Host passes xT [D, N] (transposed activations) so every matmul contracts
    along the partition dim with no on-device transposes of x.
  - qkv is computed dim-major: qT/kT/vT [128(=2 heads x 64), N].
  - RMSNorm per token over head_dim (= partition dim here) is done with
    ones-matmuls on the PE (per-head sum of squares -> [2, N]) and a
    broadcast-back matmul, then one DVE scalar_tensor_tensor multiply.
  - scores are computed transposed: sT [tok_k, tok_q]; exp() on ACT; the
    PV matmul contracts tok_k on partitions with V in natural layout
    augmented with a ones column, so row 64 of the PV accumulator is the
    softmax denominator. No max-subtraction is needed: post-RMSNorm
    |q.k|/sqrt(hd) <= sqrt(hd)*max_scale^2 (= 8 with unit scales).
  - out proj contracts 64 head-dims per head (2 accumulating matmuls per
    tile); per-core partial outputs are summed on the host (the TP
    all-reduce) together with bout.
Matmul-feeding tensors use dtype float32r (fp32 storage, reduced-precision
full-rate PE mode); everything else is fp32.
"""

import os

import numpy as np

B, S, D, H = 2, 2048, 1024, 16
HD = D // H            # 64
N = B * S              # 4096 tokens
NCORES = 8
HPC = H // NCORES      # 2 heads per core
PD = HPC * HD          # 128 per-core head dims
EPS = 1e-6
TOK_T = 512            # phase-1/2 token tile (free dim)
KB = 128               # key block (partition dim in PV)
VW = HD + 2            # vaug width (64 v dims + ones col + pad; even for fp32r)

_last_results = None   # test.py introspection (exec_time_ns, profile)
_nc_cache = None


def _build_program():
    global _nc_cache
    if _nc_cache is not None:
        return _nc_cache
    _nc_cache = _build_program_uncached()
    return _nc_cache


def _build_program_uncached():
    import concourse.bacc as bacc
    import concourse.bass as bass
    import concourse.mybir as mybir
    import concourse.tile as tile
    from concourse.masks import make_identity

    f32 = mybir.dt.float32
    f32r = mybir.dt.float32r
    AF = mybir.ActivationFunctionType
    ALU = mybir.AluOpType

    nc = bacc.Bacc(None, target_bir_lowering=False, debug=False)

    xT_h = nc.declare_dram_parameter("xT", [D, N], f32r, isOutput=False)
    Wq_h = nc.declare_dram_parameter("Wq", [D, 3 * PD], f32r, isOutput=False)
    bq_h = nc.declare_dram_parameter("bq", [PD, 3], f32, isOutput=False)
    Wo_h = nc.declare_dram_parameter("Wo", [PD, D], f32r, isOutput=False)
    qs_h = nc.declare_dram_parameter("qs", [PD, 1], f32, isOutput=False)
    ks_h = nc.declare_dram_parameter("ks", [PD, 1], f32, isOutput=False)
    sel2_h = nc.declare_dram_parameter("sel2", [2, 128], f32r, isOutput=False)
    ones2_h = nc.declare_dram_parameter("ones2", [128, 2], f32r, isOutput=False)
    onescol_h = nc.declare_dram_parameter("onescol", [128, HD], f32r, isOutput=False)
    onespad_h = nc.declare_dram_parameter("onespad", [128, 2], f32r, isOutput=False)
    out_h = nc.declare_dram_parameter("outp", [N, D], f32, isOutput=True)

    n_tt = N // TOK_T           # 8 token tiles
    n_kc = D // 128             # 8 contraction chunks for qkv proj
    n_kb = S // KB              # 16 key blocks per batch
    n_qt = S // TOK_T           # 4 query tiles per batch

    with nc.allow_low_precision(reason="fp32r matmul operands"), \
            tile.TileContext(nc) as tc:
        with (
            tc.tile_pool(name="big", bufs=1) as big,
            tc.tile_pool(name="consts", bufs=1) as consts,
        ):
            # ---- persistent SBUF tensors ----
            qnT = big.tile([PD, N], f32r, tag="qnT")
            knT = big.tile([PD, N], f32r, tag="knT")
            vT = big.tile([PD, N], f32, tag="vT")
            # oT with both heads stacked on partitions (h1 arrives via a
            # partition-shifting SBUF->SBUF DMA)
            onT = big.tile([PD, N], f32r, tag="onT")
            # v in natural layout + ones col (+pad): one [128, VW] tile per (b,h,kb)
            vaug = big.tile([KB, B * HPC * n_kb, VW], f32r, tag="vaug")
            Wsb = big.tile([128, n_kc, 3 * PD], f32r, tag="Wsb")
            WoSb = big.tile([PD, D], f32r, tag="WoSb")
            bqSb = consts.tile([PD, 3], f32, tag="bqSb")
            qsSb = consts.tile([PD, 1], f32, tag="qsSb")
            ksSb = consts.tile([PD, 1], f32, tag="ksSb")
            sel2 = consts.tile([2, 128], f32r, tag="sel2")

            Win = Wq_h[:, :].rearrange("(kc p) j -> p kc j", p=128)
            nc.sync.dma_start(out=Wsb[:, 0, :], in_=Win[:, 0, :])
            nc.sync.dma_start(out=Wsb[:, 1:n_kc, :], in_=Win[:, 1:n_kc, :])
            nc.sync.dma_start(out=WoSb, in_=Wo_h[:, :])
            nc.sync.dma_start(out=bqSb, in_=bq_h[:, :])
            nc.sync.dma_start(out=qsSb, in_=qs_h[:, :])
            nc.sync.dma_start(out=ksSb, in_=ks_h[:, :])
            nc.sync.dma_start(out=sel2, in_=sel2_h[:, :])

            # ---- constants ----
            ident = consts.tile([128, HD], f32, tag="ident")
            make_identity(nc, ident[0:HD, :])
            make_identity(nc, ident[HD:2 * HD, :])
            ones2 = consts.tile([128, 2], f32r, tag="ones2")
            nc.sync.dma_start(out=ones2, in_=ones2_h[:, :])
            ones_col = consts.tile([128, HD], f32r, tag="ones_col")
            nc.sync.dma_start(out=ones_col, in_=onescol_h[:, :])
            # fill every vaug [ones, pad] column pair with one broadcast DMA
            nc.sync.dma_start(
                out=vaug[:, :, HD:VW],
                in_=onespad_h[:, :].unsqueeze(1).broadcast_to(
                    [128, B * HPC * n_kb, 2]),
            )
            eps2 = consts.tile([2, 1], f32, tag="eps2")
            nc.vector.memset(eps2, EPS)
            zb = consts.tile([128, 1], f32, tag="zb")
            nc.vector.memset(zb, 0.0)

            # ================= Phase 1: qkvT + RMSNorm =================
            with (
                tc.tile_pool(name="p1x", bufs=3) as p1x,
                tc.tile_pool(name="p1t", bufs=3) as p1t,
                tc.tile_pool(name="p1s", bufs=4) as p1s,
                tc.tile_pool(name="ps_qkv", bufs=3, space=bass.MemorySpace.PSUM) as ps_qkv,
                tc.tile_pool(name="ps_sum", bufs=2, space=bass.MemorySpace.PSUM) as ps_sum,
                tc.tile_pool(name="ps_bc", bufs=2, space=bass.MemorySpace.PSUM) as ps_bc,
                tc.tile_pool(name="ps_tp", bufs=1, space=bass.MemorySpace.PSUM) as ps_tp,
            ):
                for t in range(n_tt):
                    tsl = slice(t * TOK_T, (t + 1) * TOK_T)
                    xt = p1x.tile([128, n_kc, TOK_T], f32r, tag="xt")
                    xin = xT_h[:, tsl].rearrange("(kc p) n -> p kc n", p=128)
                    for kc in range(n_kc):
                        nc.sync.dma_start(out=xt[:, kc, :], in_=xin[:, kc, :])
                    for m in range(3):  # 0=q, 1=k, 2=v
                        ps = ps_qkv.tile([128, TOK_T], f32, tag="ps")
                        for kc in range(n_kc):
                            nc.tensor.matmul(
                                ps,
                                Wsb[:, kc, m * 128:(m + 1) * 128],
                                xt[:, kc, :],
                                start=(kc == 0),
                                stop=(kc == n_kc - 1),
                            )
                        if m == 2:
                            nc.vector.tensor_scalar_add(vT[:, tsl], ps,
                                                        bqSb[:, 2:3])
                            b = t * TOK_T // S
                            for h in range(HPC):
                                for j in range(TOK_T // KB):
                                    tok0 = t * TOK_T + j * KB
                                    kb = (tok0 - b * S) // KB
                                    idx = (b * HPC + h) * n_kb + kb
                                    tp = ps_tp.tile([KB, HD], f32, tag="tp",
                                                    name="tp")
                                    nc.tensor.transpose(
                                        tp,
                                        vT[h * HD:(h + 1) * HD,
                                           tok0:tok0 + KB],
                                        ident[h * HD:(h + 1) * HD, :],
                                    )
                                    nc.scalar.copy(
                                        out=vaug[:, idx, 0:HD], in_=tp)
                            continue
                        raw = p1t.tile([128, TOK_T], f32, tag="raw")
                        nc.vector.tensor_scalar_add(raw, ps, bqSb[:, m:m + 1])
                        sq = p1t.tile([128, TOK_T], f32r, tag="sq")
                        nc.vector.tensor_mul(sq, raw, raw)
                        ssum = ps_sum.tile([2, TOK_T], f32, tag="ssum")
                        nc.tensor.matmul(ssum, ones2[:, :], sq[:, :],
                                         start=True, stop=True)
                        std = p1s.tile([2, TOK_T], f32, tag="std")
                        nc.scalar.activation(out=std, in_=ssum, func=AF.Sqrt,
                                             bias=eps2[:, :], scale=1.0 / HD)
                        rstd = p1s.tile([2, TOK_T], f32r, tag="rstd")
                        nc.vector.reciprocal(rstd, std)
                        bc = ps_bc.tile([128, TOK_T], f32, tag="bc")
                        nc.tensor.matmul(bc, sel2[:, :], rstd[:, :],
                                         start=True, stop=True)
                        dst = qnT if m == 0 else knT
                        sc = qsSb if m == 0 else ksSb
                        nc.vector.scalar_tensor_tensor(
                            out=dst[:, tsl], in0=raw, scalar=sc[:, 0:1], in1=bc,
                            op0=ALU.mult, op1=ALU.mult,
                        )

            # ========= Phase 2: attention + interleaved out-proj =========
            # The two heads' K=64 score matmuls live at PE row-groups 0-1 /
            # 2-3 (base partitions 0 / 64) and run concurrently in the array.
            # Scores for two consecutive key blocks share one 2-bank PSUM
            # tile so each Exp instruction covers 1024 elements. The
            # out-projection for each query tile is emitted right after its
            # normalize, overlapping the attention of later tiles.
            with (
                tc.tile_pool(name="p2p", bufs=4) as p2p,
                tc.tile_pool(name="p2s", bufs=4) as p2s,
                tc.tile_pool(name="p3o", bufs=4) as p3o,
                tc.tile_pool(name="ps_sc", bufs=2, space=bass.MemorySpace.PSUM) as ps_sc,
                tc.tile_pool(name="ps_o", bufs=1, space=bass.MemorySpace.PSUM) as ps_o,
                tc.tile_pool(name="ps_3", bufs=1, space=bass.MemorySpace.PSUM) as ps_3,
                tc.tile_pool(name="ps_b2", bufs=1, space=bass.MemorySpace.PSUM) as ps_b2,
            ):
                for b in range(B):
                    for qt in range(n_qt):
                        q0 = b * S + qt * TOK_T
                        qsl = slice(q0, q0 + TOK_T)
                        po = [ps_o.tile([VW, TOK_T], f32, tag=f"po{h}",
                                        name=f"po{h}") for h in range(HPC)]
                        for kb2 in range(n_kb // 2):
                            for h in range(HPC):
                                hsl = slice(h * HD, (h + 1) * HD)
                                pss = ps_sc.tile([KB, 2, TOK_T], f32,
                                                 tag="pss", name="pss")
                                for j in range(2):
                                    kb = kb2 * 2 + j
                                    k0 = b * S + kb * KB
                                    nc.tensor.matmul(
                                        pss[:, j, :],
                                        knT[hsl, k0:k0 + KB],
                                        qnT[hsl, qsl],
                                        start=True, stop=True,
                                    )
                                pt = p2p.tile([KB, 2, TOK_T], f32r,
                                              tag=f"pt{h}", name=f"pt{h}")
                                nc.scalar.activation(out=pt, in_=pss,
                                                     func=AF.Exp,
                                                     bias=zb[:, :], scale=1.0)
                                for j in range(2):
                                    kb = kb2 * 2 + j
                                    nc.tensor.matmul(
                                        po[h],
                                        vaug[:, (b * HPC + h) * n_kb + kb, :],
                                        pt[:, j, :],
                                        start=(kb == 0),
                                        stop=(kb == n_kb - 1),
                                    )
                        for h in range(HPC):
                            # copy PV accumulator out fast to free its bank
                            ou = p2s.tile([VW, TOK_T], f32, tag="ou")
                            nc.vector.tensor_copy(ou, po[h])
                            rec = p2s.tile([HD + 1, TOK_T], f32r, tag="rec")
                            nc.vector.reciprocal(rec[HD:HD + 1, :],
                                                 ou[HD:HD + 1, :])
                            bc1 = ps_b2.tile([HD, TOK_T], f32, tag="bc1")
                            nc.tensor.matmul(bc1,
                                             ones_col[HD:HD + 1, :],
                                             rec[HD:HD + 1, :],
                                             start=True, stop=True)
                            bc1s = p2s.tile([HD, TOK_T], f32, tag="bc1s")
                            nc.vector.tensor_copy(bc1s, bc1)
                            if h == 0:
                                nc.vector.tensor_mul(onT[0:HD, qsl],
                                                     ou[0:HD, :], bc1s)
                            else:
                                oh1 = p2s.tile([HD, TOK_T], f32r, tag="oh1")
                                nc.vector.tensor_mul(oh1, ou[0:HD, :], bc1s)
                                nc.sync.dma_start(out=onT[HD:PD, qsl],
                                                  in_=oh1)
                        # out-projection for this query tile
                        last_qt = (b == B - 1 and qt == n_qt - 1)
                        for tb in range(q0 // 128, (q0 + TOK_T) // 128):
                            for od in range(D // TOK_T):
                                i3 = tb * 2 + od
                                if last_qt and i3 % 2 == 1:
                                    # PV accumulators are retired; reuse
                                    # their banks to double-buffer the tail
                                    ps3 = ps_o.tile([128, TOK_T], f32,
                                                    tag=f"po{i3 % 4 // 2}",
                                                    name="ps3t")
                                else:
                                    ps3 = ps_3.tile([128, TOK_T], f32,
                                                    tag="ps3", name="ps3")
                                nc.tensor.matmul(
                                    ps3,
                                    onT[:, tb * 128:(tb + 1) * 128],
                                    WoSb[:, od * TOK_T:(od + 1) * TOK_T],
                                    start=True, stop=True,
                                )
                                ot = p3o.tile([128, TOK_T], f32, tag="ot")
                                nc.vector.tensor_copy(ot, ps3)
                                nc.sync.dma_start(
                                    out=out_h[tb * 128:(tb + 1) * 128,
                                              od * TOK_T:(od + 1) * TOK_T],
                                    in_=ot,
                                )

    nc.compile()
    return nc


def kernel(x, Wqkv, bqkv, Wout, bout, q_scale, k_scale):
    global _last_results
    from concourse.bass_utils import run_bass_kernel_spmd

    x = np.asarray(x, dtype=np.float32)
    Wqkv = np.asarray(Wqkv, dtype=np.float32)
    bqkv = np.asarray(bqkv, dtype=np.float32)
    Wout = np.asarray(Wout, dtype=np.float32)
    bout = np.asarray(bout, dtype=np.float32)
    q_scale = np.asarray(q_scale, dtype=np.float32)
    k_scale = np.asarray(k_scale, dtype=np.float32)

    xT = np.ascontiguousarray(x.reshape(N, D).T)
    sel2 = np.zeros((2, 128), dtype=np.float32)
    sel2[0, 0:64] = 1.0
    sel2[1, 64:128] = 1.0
    in_maps = []
    for c in range(NCORES):
        c0 = c * PD
        Wq_s = np.ascontiguousarray(np.concatenate(
            [Wqkv[:, c0:c0 + PD], Wqkv[:, D + c0:D + c0 + PD],
             Wqkv[:, 2 * D + c0:2 * D + c0 + PD]], axis=1))
        bq_s = np.ascontiguousarray(np.stack(
            [bqkv[c0:c0 + PD], bqkv[D + c0:D + c0 + PD],
             bqkv[2 * D + c0:2 * D + c0 + PD]], axis=1))
        Wo_s = np.ascontiguousarray(Wout[c0:c0 + PD, :])
        qs2 = np.ascontiguousarray(np.tile(q_scale, HPC).reshape(PD, 1) / np.sqrt(HD))
        ks2 = np.ascontiguousarray(np.tile(k_scale, HPC).reshape(PD, 1))
        ones2 = np.zeros((128, 2), dtype=np.float32)
        ones2[0:64, 0] = 1.0
        ones2[64:128, 1] = 1.0
        onescol = np.ones((128, HD), dtype=np.float32)
        onespad = np.zeros((128, 2), dtype=np.float32)
        onespad[:, 0] = 1.0
        in_maps.append({"xT": xT, "Wq": Wq_s, "bq": bq_s, "Wo": Wo_s,
                        "qs": qs2.astype(np.float32), "ks": ks2.astype(np.float32),
                        "sel2": sel2, "ones2": ones2, "onescol": onescol,
                        "onespad": onespad})

    nc = _build_program()
    res = run_bass_kernel_spmd(nc, in_maps, core_ids=list(range(NCORES)))
    _last_results = res

    acc = res.results[0]["outp"].astype(np.float32)
    for c in range(1, NCORES):
        acc = acc + res.results[c]["outp"]
    acc = acc + bout
    return acc.reshape(B, S, D).astype(np.float32)



# revision 35
# speedup vs baseline: 1.2193x; 1.2193x over previous
"""Fused multi-head attention block (qkv proj + RMSNorm(q,k) + softmax(QK^T)V
+ out proj), tensor-parallel over 8 TRN2 NeuronCores (2 heads per core).

Cost-model-driven design:
  - All matmul operands bf16 (1 cycle/row at any free size).
  - PV computed transposed: o[q, hd] += P[k, q]^T v[k, hd] with V as the
    small moving operand (F=66 incl. a ones-column that accumulates the
    softmax denominator) -> ~2x cheaper than the natural orientation, and
    the denominator becomes a per-partition scalar for the DVE.
  - v projected directly into natural [token, hd] layout (x stationary).
  - k_scale folded into the q-side scale on the host (scores are bilinear).
  - RMS statistics computed transposed ([token, head] via tiny F=2 matmuls)
    so rstd = exp(-0.5*ln(msq+eps)) runs on 16-element ACT ops; the whole
    kernel then uses ONE ACT table (Ln+Exp) -> no table reloads. rstd is
    broadcast back over head dims by an identity matmul with a stride-0
    weights view.
  - Scores accumulate in double-buffered 2-bank PSUM tiles; exp() runs on
    [128, 1024] elements per instruction. PSUM budget (8 banks): score
    tiles 2x2 + two PV accumulators + two rotating scratch banks.
  - Only one pending PSUM accumulation group per 2KB bank is legal on HW
    (start=True resets the bank's has_written bits), so each PV qs-slice
    runs as one complete 16-block group; exp outputs are buffered deep.
  - A labeled work-queue feeds phase-1 (projections/norm) and out-proj
    pieces into the attention unit loops in ~1us chunks with explicit
    producer->consumer drain barriers, so the in-order engine queues never
    head-block and the PE stays dense. Paired tiles' chains interleave so
    one tile's projections hide the other's cross-engine norm latency;
    batch-0 out-projections are deferred into the ACT-bound batch-1 units.
  - o[q, hd] transposed back on the PE (bf16, 1 c/r) for the out proj.
  - Per-core partial outputs written bf16; host sums in fp32 (+bout).
"""

import numpy as np

B, S, D, H = 2, 2048, 1024, 16
HD = D // H            # 64
N = B * S              # 4096 tokens
NCORES = 8
HPC = H // NCORES      # 2 heads per core
PD = HPC * HD          # 128 per-core head dims
EPS = 1e-6
TOK_T = 512            # token tile / query tile
KB = 128               # key block (contraction dim of PV)
VW = HD + 2            # per-head v width (64 dims + ones col + pad)

_last_results = None   # test.py introspection
_nc_cache = {}
_last_flag = [False]


def _build_program(has_qkv_bias=None):
    if has_qkv_bias is None:
        has_qkv_bias = _last_flag[0]
    if has_qkv_bias not in _nc_cache:
        _nc_cache[has_qkv_bias] = _build_program_uncached(has_qkv_bias)
    _last_flag[0] = has_qkv_bias
    return _nc_cache[has_qkv_bias]


def _build_program_uncached(has_qkv_bias):
    import concourse.bacc as bacc
    import concourse.bass as bass
    import concourse.mybir as mybir
    import concourse.tile as tile
    from concourse.masks import make_identity

    f32 = mybir.dt.float32
    bf16 = mybir.dt.bfloat16
    AF = mybir.ActivationFunctionType
    ALU = mybir.AluOpType

    nc = bacc.Bacc(None, target_bir_lowering=False, debug=False)

    xT_h = nc.declare_dram_parameter("xT", [D, N], bf16, isOutput=False)
    Wq_h = nc.declare_dram_parameter("Wq", [D, 3 * PD], bf16, isOutput=False)
    bq_h = nc.declare_dram_parameter("bq", [PD, 3], f32, isOutput=False)
    bv_h = nc.declare_dram_parameter("bv", [1, PD], bf16, isOutput=False)
    Wo_h = nc.declare_dram_parameter("Wo", [PD, D], bf16, isOutput=False)
    qks_h = nc.declare_dram_parameter("qks", [PD, 1], f32, isOutput=False)
    out_h = nc.declare_dram_parameter("outp", [N, D], bf16, isOutput=True)

    n_kc = D // 128             # 8 contraction chunks for qkv proj
    n_kb = S // KB              # 16 key blocks per batch
    NG = 2                      # kb per score group (2-bank psum tile, x2 bufs)

    # Force one ACT function table (Ln+Exp) for the whole program: pass the
    # table list with every other set emptied (indices preserved so
    # act_func_set_id still points into act_info.json's act_func_sets).
    import types as _types
    from concourse.hw_specs import get_activation_tables as _gat

    def _one_table_loads(self):
        import bass_rust as _bass_rust
        has_activation = any(
            isinstance(i, mybir.InstActivation)
            for b in self.main_func.blocks
            for i in b.instructions
        )
        if not has_activation:
            return
        tabs = [
            (name, (funcs if name == "natural_log_exp_and_others" else set()))
            for name, funcs in _gat(self.m.arch).items()
        ]
        _bass_rust.insert_act_table_loads(self, tabs)

    nc.insert_act_table_loads = _types.MethodType(_one_table_loads, nc)

    with nc.allow_low_precision(reason="bf16 matmul pipeline"), \
            tile.TileContext(nc) as tc:
        with (
            tc.tile_pool(name="big", bufs=1) as big,
            tc.tile_pool(name="consts", bufs=1) as consts,
            tc.tile_pool(name="p1s", bufs=3) as p1s,
            tc.tile_pool(name="ptp", bufs=20) as ptp,
            tc.tile_pool(name="onp", bufs=2) as onp,
            tc.tile_pool(name="onTp", bufs=6) as onTp,
            tc.tile_pool(name="otp", bufs=4) as otp,
            tc.tile_pool(name="recp", bufs=2) as recp,
            tc.tile_pool(name="ps4", bufs=2, space=bass.MemorySpace.PSUM) as ps4,
            tc.tile_pool(name="ps_o", bufs=1, space=bass.MemorySpace.PSUM) as ps_o,
            tc.tile_pool(name="ps_x", bufs=2, space=bass.MemorySpace.PSUM) as ps_x,
        ):
            # ---- persistent SBUF tensors ----
            xsb = big.tile([128, n_kc, N], bf16, tag="xsb")
            qnT = big.tile([PD, N], bf16, tag="qnT")
            knT = big.tile([PD, N], bf16, tag="knT")
            vN = big.tile([KB, N // KB, 2 * VW], bf16, tag="vN")
            Wsb = big.tile([128, n_kc, 3 * PD], bf16, tag="Wsb")
            WoSb = big.tile([PD, D], bf16, tag="WoSb")

            bqSb = consts.tile([PD, 3], f32, tag="bqSb")
            bvSb = consts.tile([1, PD], bf16, tag="bvSb")
            qksSb = consts.tile([PD, 1], f32, tag="qksSb")
            ones2 = consts.tile([128, 2], bf16, tag="ones2")
            ident = consts.tile([128, 128], bf16, tag="ident")
            onesrow = consts.tile([1, KB], bf16, tag="onesrow")
            eps128 = consts.tile([128, 1], f32, tag="eps128")
            zb128 = consts.tile([128, 1], f32, tag="zb128")

            Win = Wq_h[:, :].rearrange("(kc p) j -> p kc j", p=128)
            xin = xT_h[:, :].rearrange("(kc p) n -> p kc n", p=128)

            # early DMAs, ordered so tile-0 chains can start ASAP
            def dma_x(t):
                tsl = slice(t * TOK_T, (t + 1) * TOK_T)
                nc.sync.dma_start(out=xsb[:, 0:4, tsl], in_=xin[:, 0:4, tsl])
                nc.sync.dma_start(out=xsb[:, 4:8, tsl], in_=xin[:, 4:8, tsl])

            nc.sync.dma_start(out=bqSb, in_=bq_h[:, :])
            nc.sync.dma_start(out=qksSb, in_=qks_h[:, :])
            nc.sync.dma_start(out=bvSb, in_=bv_h[:, :])
            nc.sync.dma_start(out=xsb[:, 0:4, 0:TOK_T], in_=xin[:, 0:4, 0:TOK_T])
            nc.sync.dma_start(out=Wsb[:, :, 128:256], in_=Win[:, :, 128:256])
            nc.sync.dma_start(out=xsb[:, 4:8, 0:TOK_T], in_=xin[:, 4:8, 0:TOK_T])
            nc.sync.dma_start(out=Wsb[:, :, 0:128], in_=Win[:, :, 0:128])
            dma_x(1)
            nc.sync.dma_start(out=Wsb[:, :, 256:384], in_=Win[:, :, 256:384])
            for t in range(2, 4):
                dma_x(t)
            nc.sync.dma_start(out=WoSb, in_=Wo_h[:, :])
            for t in range(4, 8):
                dma_x(t)
            # PE p-state warm-up: harmless matmuls on a zeroed tile keep the
            # tensor engine ramping to full clock while the first DMAs land
            warm = consts.tile([128, TOK_T], bf16, tag="warm")
            nc.vector.memset(warm, 0.0)
            for w in range(12):
                pw = ps_o.tile([KB, TOK_T], f32, tag="po0", name="pw")
                nc.tensor.matmul(pw, warm[:, 0:128], warm,
                                 start=True, stop=True)
            make_identity(nc, ident)
            nc.vector.memset(onesrow, 1.0)
            nc.vector.memset(eps128, EPS)
            nc.vector.memset(zb128, 0.0)
            nc.vector.memset(ones2, 0.0)
            nc.vector.memset(ones2[0:64, 0:1], 1.0)
            nc.vector.memset(ones2[64:128, 1:2], 1.0)
            # vN ones + pad columns (softmax denominator trick)
            nc.vector.memset(vN[:, :, HD:HD + 1], 1.0)
            nc.vector.memset(vN[:, :, HD + 1:HD + 2], 0.0)
            nc.vector.memset(vN[:, :, VW + HD:VW + HD + 1], 1.0)
            nc.vector.memset(vN[:, :, VW + HD + 1:VW + HD + 2], 0.0)

            # ---------------- phase-1 pieces ----------------
            tile_state = {}

            def p_qkproj(t, m):
                # q (m=0) or k (m=1) projection; cast+bias; square
                tsl = slice(t * TOK_T, (t + 1) * TOK_T)
                px = ps_x.tile([128, TOK_T], f32, tag="px", name="pxqk")
                for kc in range(n_kc):
                    nc.tensor.matmul(
                        px,
                        Wsb[:, kc, m * 128:(m + 1) * 128],
                        xsb[:, kc, tsl],
                        start=(kc == 0),
                        stop=(kc == n_kc - 1),
                    )
                raw = p1s.tile([128, TOK_T], bf16, tag=f"raw{m}",
                               name=f"raw{m}")
                nc.vector.tensor_scalar_add(raw, px, bqSb[:, m:m + 1])
                st = tile_state.setdefault(t, {})
                if "sq" not in st:
                    st["sq"] = p1s.tile([128, 2, TOK_T], bf16, tag="sq",
                                        name="sq", bufs=2)
                nc.vector.tensor_mul(st["sq"][:, m, :], raw, raw)
                st[f"raw{m}"] = raw

            def p_norm(t):
                # transposed mean-square stats + rstd = exp(-0.5 ln(ms+eps))
                st = tile_state[t]
                sq = st["sq"]
                msx = ps_x.tile([128, TOK_T], f32, tag="px", name="msx")
                msv = msx[:, 0:16].rearrange("p (c m h) -> p c m h", c=4, m=2)
                for c in range(4):
                    for m in range(2):
                        nc.tensor.matmul(
                            msv[:, c, m, :],
                            sq[:, m, c * 128:(c + 1) * 128],
                            ones2,
                            start=True, stop=True)
                lnv = p1s.tile([128, 16], f32, tag="lnv", name="lnv")
                nc.scalar.activation(out=lnv, in_=msx[:, 0:16], func=AF.Ln,
                                     bias=eps128[:, :], scale=1.0 / HD)
                rstdT = p1s.tile([128, 4, 2, 2], bf16, tag="rstdT",
                                 name="rstdT")
                nc.scalar.activation(
                    out=rstdT.rearrange("p c m h -> p (c m h)"), in_=lnv,
                    func=AF.Exp, bias=zb128[:, :], scale=-0.5)
                st["rstdT"] = rstdT

            def p_qkout(t, m):
                # broadcast rstd over head dims: materialize [tok, hd] rows
                # on the DVE (stride-0 read), then transpose via identity
                # matmul; normalize on the DVE
                st = tile_state[t]
                tsl = slice(t * TOK_T, (t + 1) * TOK_T)
                bcT = p1s.tile([128, 4, 128], bf16, tag="bcT", name="bcT")
                nc.vector.tensor_copy(
                    bcT.rearrange("p c (h x) -> p c h x", h=2),
                    st["rstdT"][:, :, m, :].unsqueeze(3).broadcast_to(
                        [128, 4, 2, HD]))
                bc = ps_x.tile([128, TOK_T], f32, tag="px", name="bc")
                for c in range(4):
                    nc.tensor.matmul(bc[:, c * 128:(c + 1) * 128],
                                     bcT[:, c, :], ident,
                                     start=True, stop=True)
                if m == 0:
                    nc.vector.scalar_tensor_tensor(
                        out=qnT[:, tsl], in0=st["raw0"],
                        scalar=qksSb[:, 0:1], in1=bc,
                        op0=ALU.mult, op1=ALU.mult)
                else:
                    nc.vector.tensor_mul(knT[:, tsl], st["raw1"], bc)

            def p_v(t):
                # v in natural [token, hd] layout (x stationary)
                pv = ps_x.tile([128, 4, 128], f32, tag="px", name="pv")
                for j in range(4):
                    tok0 = t * TOK_T + j * 128
                    for kc in range(n_kc):
                        nc.tensor.matmul(
                            pv[:, j, :],
                            xsb[:, kc, tok0:tok0 + 128],
                            Wsb[:, kc, 256:384],
                            start=(kc == 0),
                            stop=(kc == n_kc - 1) and not has_qkv_bias,
                        )
                    if has_qkv_bias:
                        nc.tensor.matmul(pv[:, j, :], onesrow[:, 0:128], bvSb,
                                         start=False, stop=True)
                c0 = t * 4
                dst = vN[:, c0:c0 + 4, :].rearrange(
                    "p c (h x) -> p c h x", h=2)[:, :, :, 0:HD]
                src = pv.rearrange("p c (h x) -> p c h x", h=2)
                nc.vector.tensor_copy(dst, src)

            # ---------------- work queue ----------------
            work = []
            pending_labels = set()

            def push(cost, fn, label=None):
                if label is not None:
                    pending_labels.add(label)
                work.append((cost, fn, label))

            def _pop_one():
                c, fn, label = work.pop(0)
                fn()
                if label is not None:
                    pending_labels.discard(label)
                return c

            def feed(budget):
                spent = 0
                while work and spent < budget:
                    spent += _pop_one()

            def drain_through(label):
                # emit queued pieces until `label` has been emitted; no-op if
                # the label was never pushed or already done
                while label in pending_labels:
                    _pop_one()

            def push_chain(t):
                # order chosen so PE-independent work (v proj) separates the
                # chain's cross-engine dependencies (cast/sq -> msq -> rstd)
                push(1800, lambda: p_qkproj(t, 1))
                push(1800, lambda: p_qkproj(t, 0))
                push(700, lambda: p_norm(t))
                push(700, lambda: p_qkout(t, 1), ("kout", t))
                push(700, lambda: p_qkout(t, 0), ("qout", t))
                push(2000, lambda: p_v(t), ("v", t))

            def push_chain_pair(a, b):
                # interleave two tiles' chains: tile b's projections hide
                # tile a's cross-engine (DVE/ACT) normalization latency
                push(1800, lambda: p_qkproj(a, 1))
                push(1800, lambda: p_qkproj(a, 0))
                push(1800, lambda: p_qkproj(b, 1))
                push(700, lambda: p_norm(a))
                push(1800, lambda: p_qkproj(b, 0))
                push(700, lambda: p_qkout(a, 1), ("kout", a))
                push(700, lambda: p_norm(b))
                push(700, lambda: p_qkout(a, 0), ("qout", a))
                push(700, lambda: p_qkout(b, 1), ("kout", b))
                push(2000, lambda: p_v(a), ("v", a))
                push(700, lambda: p_qkout(b, 0), ("qout", b))
                push(2000, lambda: p_v(b), ("v", b))

            # ---------------- attention unit ----------------
            uid_ctr = [0]

            def emit_unit(b, qt, last=False, next_q=None, defer_ops=False):
                uid = uid_ctr[0]
                uid_ctr[0] += 1
                q0 = b * S + qt * TOK_T
                qsl = slice(q0, q0 + TOK_T)
                drain_through(("qout", b * 4 + qt))
                onT = onTp.tile([PD, TOK_T], bf16, tag="onT", name="onT")
                po = {}
                pov = {}
                for h in range(HPC):
                    po[h] = ps_o.tile([KB, TOK_T], f32, tag=f"po{h}",
                                      name=f"po{h}")
                    pov[h] = po[h][:, 0:4 * VW].rearrange(
                        "p (a b) -> p a b", a=4)

                def finish_head(h):
                    hsl = slice(h * HD, (h + 1) * HD)

                    def norm():
                        rec = recp.tile([128, 4, 1], f32, tag="rec",
                                        name="rec")
                        nc.vector.reciprocal(rec, pov[h][:, :, HD:HD + 1])
                        o_n = onp.tile([128, 4, HD], bf16, tag=f"on{h}",
                                       name=f"on{h}")
                        nc.vector.tensor_mul(
                            o_n, pov[h][:, :, 0:HD],
                            rec.broadcast_to([128, 4, HD]))
                        tile_state[("on", h)] = o_n

                    def transp():
                        o_n = tile_state.pop(("on", h))
                        tp = ps_x.tile([HD, 4, 128], bf16, tag="px",
                                       name="tp")
                        for qs in range(4):
                            nc.tensor.transpose(tp[:, qs, :], o_n[:, qs, :],
                                                ident)
                        nc.vector.tensor_copy(
                            onT[hsl, :].rearrange("p (a c) -> p a c", a=4),
                            tp)

                    push(500, norm, ("norm", uid, h))
                    push(800, transp)

                def pv_chunk(pts_h, h, qs):
                    # one full accumulation group (all 16 kb) per qs slice:
                    # only one pending PSUM group per bank is legal
                    for kb in range(n_kb):
                        g, j = kb // NG, kb % NG
                        nc.tensor.matmul(
                            pov[h][:, qs, 0:HD + 1],
                            pts_h[g][:, j, qs * 128:(qs + 1) * 128],
                            vN[:, b * n_kb + kb, h * VW:h * VW + HD + 1],
                            start=(kb == 0),
                            stop=(kb == n_kb - 1),
                        )

                h_order = (1, 0) if last else (0, 1)
                pts = {h: [] for h in h_order}
                prev_h = None
                for h in h_order:
                    for g in range(n_kb // NG):
                        drain_through(("kout", b * 4 + g // 2))
                        hsl = slice(h * HD, (h + 1) * HD)
                        pss = ps4.tile([KB, NG, TOK_T], f32, tag="pss",
                                       name="pss")
                        for j in range(NG):
                            kb = g * NG + j
                            k0 = b * S + kb * KB
                            nc.tensor.matmul(pss[:, j, :],
                                             knT[hsl, k0:k0 + KB],
                                             qnT[hsl, qsl],
                                             start=True, stop=True)
                        pt = ptp.tile([KB, NG, TOK_T], bf16, tag="pt",
                                      name="pt")
                        nc.scalar.activation(out=pt, in_=pss, func=AF.Exp,
                                             bias=zb128[:, :], scale=1.0)
                        pts[h].append(pt)
                        if prev_h is not None and g % 2 == 1:
                            if g == 1:
                                drain_through(("v", b * 4 + 3))
                                if uid > 0:
                                    drain_through(("norm", uid - 1, prev_h))
                            pv_chunk(pts[prev_h], prev_h, g // 2)
                            if g == n_kb // NG - 1:
                                finish_head(prev_h)
                        if prev_h is not None and g == 4 and next_q is not None:
                            drain_through(next_q)
                        if prev_h is not None and g == 6:
                            drain_through(("kout", (b * 4 + 4) % 8))
                        feed(1100)
                    prev_h = h
                if uid > 0:
                    drain_through(("norm", uid - 1, prev_h))
                for qs in range(4):
                    push(600, (lambda qs=qs: pv_chunk(pts[prev_h], prev_h,
                                                      qs)))
                finish_head(prev_h)

                ot_ref = {}
                op_pieces = []
                for tb in range(4):
                    for od in range(2):
                        def oph(tb=tb, od=od):
                            if od == 0:
                                ot_ref[tb] = otp.tile([128, D], bf16,
                                                      tag="ot", name="ot")
                            ot = ot_ref[tb]
                            px = ps_x.tile([128, TOK_T], f32, tag="px",
                                           name="pxop")
                            nc.tensor.matmul(
                                px,
                                onT[:, tb * 128:(tb + 1) * 128],
                                WoSb[:, od * TOK_T:(od + 1) * TOK_T],
                                start=True, stop=True)
                            if last and od == 1:
                                nc.scalar.copy(
                                    out=ot[:, od * TOK_T:(od + 1) * TOK_T],
                                    in_=px)
                            else:
                                nc.vector.tensor_copy(
                                    ot[:, od * TOK_T:(od + 1) * TOK_T], px)
                            if od == 1:
                                r0 = q0 + tb * 128
                                nc.sync.dma_start(
                                    out=out_h[r0:r0 + 128, :], in_=ot)
                        op_pieces.append(oph)
                if defer_ops:
                    return op_pieces
                for p in op_pieces:
                    push(600, p)

            # ---------------- emission schedule ----------------
            # head: interleaved chains for tiles 0-1 (v deferred to feeds)
            p_qkproj(0, 1)
            p_qkproj(0, 0)
            p_qkproj(1, 1)
            p_norm(0)
            p_qkproj(1, 0)
            p_qkout(0, 1)
            p_norm(1)
            p_qkout(0, 0)
            p_qkout(1, 1)
            p_qkout(1, 0)
            push(2000, lambda: p_v(0), ("v", 0))
            push(2000, lambda: p_v(1), ("v", 1))
            push_chain_pair(2, 3)
            dops = []
            dops.append(emit_unit(0, 0, next_q=("qout", 1), defer_ops=True))
            push_chain_pair(4, 5)
            dops.append(emit_unit(0, 1, next_q=("qout", 2), defer_ops=True))
            dops.append(emit_unit(0, 2, next_q=("qout", 3), defer_ops=True))
            push_chain_pair(6, 7)
            dops.append(emit_unit(0, 3, next_q=("qout", 4), defer_ops=True))
            for p in dops[0]:
                push(600, p)
            emit_unit(1, 0, next_q=("qout", 5))
            for p in dops[1]:
                push(600, p)
            emit_unit(1, 1, next_q=("qout", 6))
            for p in dops[2]:
                push(600, p)
            emit_unit(1, 2, next_q=("qout", 7))
            for p in dops[3]:
                push(600, p)
            emit_unit(1, 3, last=True)
            feed(10 ** 9)  # drain

    nc.compile()
    return nc


def kernel(x, Wqkv, bqkv, Wout, bout, q_scale, k_scale):
    global _last_results
    import ml_dtypes
    from concourse.bass_utils import run_bass_kernel_spmd

    bf16 = ml_dtypes.bfloat16
    x = np.asarray(x, dtype=np.float32)
    Wqkv = np.asarray(Wqkv, dtype=np.float32)
    bqkv = np.asarray(bqkv, dtype=np.float32)
    Wout = np.asarray(Wout, dtype=np.float32)
    bout = np.asarray(bout, dtype=np.float32)
    q_scale = np.asarray(q_scale, dtype=np.float32)
    k_scale = np.asarray(k_scale, dtype=np.float32)

    has_qkv_bias = bool(np.any(bqkv != 0.0))

    xT = np.ascontiguousarray(x.reshape(N, D).T).astype(bf16)
    # k_scale folds into the q side (scores are bilinear in q, k)
    qks = (np.tile(q_scale * k_scale, HPC).reshape(PD, 1)
           / np.sqrt(HD)).astype(np.float32)

    in_maps = []
    for c in range(NCORES):
        c0 = c * PD
        Wq_s = np.concatenate(
            [Wqkv[:, c0:c0 + PD], Wqkv[:, D + c0:D + c0 + PD],
             Wqkv[:, 2 * D + c0:2 * D + c0 + PD]], axis=1).astype(bf16)
        bq_s = np.ascontiguousarray(np.stack(
            [bqkv[c0:c0 + PD], bqkv[D + c0:D + c0 + PD],
             bqkv[2 * D + c0:2 * D + c0 + PD]], axis=1))
        bv_s = bqkv[2 * D + c0:2 * D + c0 + PD].reshape(1, PD).astype(bf16)
        Wo_s = np.ascontiguousarray(Wout[c0:c0 + PD, :]).astype(bf16)
        in_maps.append({"xT": xT, "Wq": Wq_s, "bq": bq_s, "bv": bv_s,
                        "Wo": Wo_s, "qks": qks})

    nc = _build_program(has_qkv_bias)
    res = run_bass_kernel_spmd(nc, in_maps, core_ids=list(range(NCORES)))
    _last_results = res

    acc = res.results[0]["outp"].astype(np.float32)
    for c in range(1, NCORES):
        acc = acc + res.results[c]["outp"].astype(np.float32)
    acc = acc + bout
    return acc.reshape(B, S, D).astype(np.float32)


# revision 37
# speedup vs baseline: 1.2387x; 1.0159x over previous
"""Fused multi-head attention block (qkv proj + RMSNorm(q,k) + softmax(QK^T)V
+ out proj), tensor-parallel over 8 TRN2 NeuronCores (2 heads per core).

Cost-model-driven design:
  - All matmul operands bf16 (1 cycle/row at any free size).
  - PV computed transposed: o[q, hd] += P[k, q]^T v[k, hd] with V as the
    small moving operand (F=66 incl. a ones-column that accumulates the
    softmax denominator) -> ~2x cheaper than the natural orientation, and
    the denominator becomes a per-partition scalar for the DVE.
  - v projected directly into natural [token, hd] layout (x stationary).
  - k_scale folded into the q-side scale on the host (scores are bilinear).
  - RMS statistics computed transposed ([token, head] via tiny F=2 matmuls)
    so rstd = exp(-0.5*ln(msq+eps)) runs on 16-element ACT ops; the whole
    kernel then uses ONE ACT table (Ln+Exp) -> no table reloads. rstd is
    broadcast back over head dims by an identity matmul with a stride-0
    weights view.
  - Scores accumulate in double-buffered 2-bank PSUM tiles; exp() runs on
    [128, 1024] elements per instruction. PSUM budget (8 banks): score
    tiles 2x2 + two PV accumulators + two rotating scratch banks.
  - Only one pending PSUM accumulation group per 2KB bank is legal on HW
    (start=True resets the bank's has_written bits), so each PV qs-slice
    runs as one complete 16-block group; exp outputs are buffered deep.
  - A labeled work-queue feeds phase-1 (projections/norm) and out-proj
    pieces into the attention unit loops in ~1us chunks with explicit
    producer->consumer drain barriers, so the in-order engine queues never
    head-block and the PE stays dense. Paired tiles' chains interleave so
    one tile's projections hide the other's cross-engine norm latency;
    batch-0 out-projections are deferred into the ACT-bound batch-1 units.
  - o[q, hd] transposed back on the PE (bf16, 1 c/r) for the out proj.
  - Per-core partial outputs written bf16; host sums in fp32 (+bout).
"""

import numpy as np

B, S, D, H = 2, 2048, 1024, 16
HD = D // H            # 64
N = B * S              # 4096 tokens
NCORES = 8
HPC = H // NCORES      # 2 heads per core
PD = HPC * HD          # 128 per-core head dims
EPS = 1e-6
TOK_T = 512            # token tile / query tile
KB = 128               # key block (contraction dim of PV)
VW = HD + 2            # per-head v width (64 dims + ones col + pad)

_last_results = None   # test.py introspection
_nc_cache = {}
_last_flag = [False]


def _build_program(has_qkv_bias=None):
    if has_qkv_bias is None:
        has_qkv_bias = _last_flag[0]
    if has_qkv_bias not in _nc_cache:
        _nc_cache[has_qkv_bias] = _build_program_uncached(has_qkv_bias)
    _last_flag[0] = has_qkv_bias
    return _nc_cache[has_qkv_bias]


def _build_program_uncached(has_qkv_bias):
    import concourse.bacc as bacc
    import concourse.bass as bass
    import concourse.mybir as mybir
    import concourse.tile as tile
    from concourse.masks import make_identity

    f32 = mybir.dt.float32
    bf16 = mybir.dt.bfloat16
    AF = mybir.ActivationFunctionType
    ALU = mybir.AluOpType

    nc = bacc.Bacc(None, target_bir_lowering=False, debug=False)

    xT_h = nc.declare_dram_parameter("xT", [D, N], bf16, isOutput=False)
    Wq_h = nc.declare_dram_parameter("Wq", [D, 3 * PD], bf16, isOutput=False)
    bq_h = nc.declare_dram_parameter("bq", [PD, 3], f32, isOutput=False)
    bv_h = nc.declare_dram_parameter("bv", [1, PD], bf16, isOutput=False)
    Wo_h = nc.declare_dram_parameter("Wo", [PD, D], bf16, isOutput=False)
    qks_h = nc.declare_dram_parameter("qks", [PD, 1], f32, isOutput=False)
    out_h = nc.declare_dram_parameter("outp", [N, D], bf16, isOutput=True)

    n_kc = D // 128             # 8 contraction chunks for qkv proj
    n_kb = S // KB              # 16 key blocks per batch
    NG = 2                      # kb per score group (2-bank psum tile, x2 bufs)

    # Force one ACT function table (Ln+Exp) for the whole program: pass the
    # table list with every other set emptied (indices preserved so
    # act_func_set_id still points into act_info.json's act_func_sets).
    import types as _types
    from concourse.hw_specs import get_activation_tables as _gat

    def _one_table_loads(self):
        import bass_rust as _bass_rust
        has_activation = any(
            isinstance(i, mybir.InstActivation)
            for b in self.main_func.blocks
            for i in b.instructions
        )
        if not has_activation:
            return
        tabs = [
            (name, (funcs if name == "natural_log_exp_and_others" else set()))
            for name, funcs in _gat(self.m.arch).items()
        ]
        _bass_rust.insert_act_table_loads(self, tabs)

    nc.insert_act_table_loads = _types.MethodType(_one_table_loads, nc)

    with nc.allow_low_precision(reason="bf16 matmul pipeline"), \
            tile.TileContext(nc) as tc:
        with (
            tc.tile_pool(name="big", bufs=1) as big,
            tc.tile_pool(name="consts", bufs=1) as consts,
            tc.tile_pool(name="p1s", bufs=3) as p1s,
            tc.tile_pool(name="ptp", bufs=20) as ptp,
            tc.tile_pool(name="onp", bufs=2) as onp,
            tc.tile_pool(name="onTp", bufs=6) as onTp,
            tc.tile_pool(name="otp", bufs=4) as otp,
            tc.tile_pool(name="recp", bufs=2) as recp,
            tc.tile_pool(name="ps4", bufs=2, space=bass.MemorySpace.PSUM) as ps4,
            tc.tile_pool(name="ps_o", bufs=1, space=bass.MemorySpace.PSUM) as ps_o,
            tc.tile_pool(name="ps_x", bufs=2, space=bass.MemorySpace.PSUM) as ps_x,
        ):
            # ---- persistent SBUF tensors ----
            xsb = big.tile([128, n_kc, N], bf16, tag="xsb")
            qnT = big.tile([PD, N], bf16, tag="qnT")
            knT = big.tile([PD, N], bf16, tag="knT")
            vN = big.tile([KB, N // KB, 2 * VW], bf16, tag="vN")
            Wsb = big.tile([128, n_kc, 3 * PD], bf16, tag="Wsb")
            WoSb = big.tile([PD, D], bf16, tag="WoSb")

            bqSb = consts.tile([PD, 3], f32, tag="bqSb")
            bvSb = consts.tile([1, PD], bf16, tag="bvSb")
            qksSb = consts.tile([PD, 1], f32, tag="qksSb")
            ones2 = consts.tile([128, 2], bf16, tag="ones2")
            ident = consts.tile([128, 128], bf16, tag="ident")
            onesrow = consts.tile([1, KB], bf16, tag="onesrow")
            eps128 = consts.tile([128, 1], f32, tag="eps128")
            zb128 = consts.tile([128, 1], f32, tag="zb128")

            Win = Wq_h[:, :].rearrange("(kc p) j -> p kc j", p=128)
            xin = xT_h[:, :].rearrange("(kc p) n -> p kc n", p=128)

            # early DMAs, ordered so tile-0 chains can start ASAP
            def dma_x(t):
                tsl = slice(t * TOK_T, (t + 1) * TOK_T)
                nc.sync.dma_start(out=xsb[:, 0:4, tsl], in_=xin[:, 0:4, tsl])
                nc.sync.dma_start(out=xsb[:, 4:8, tsl], in_=xin[:, 4:8, tsl])

            nc.sync.dma_start(out=bqSb, in_=bq_h[:, :])
            nc.sync.dma_start(out=qksSb, in_=qks_h[:, :])
            nc.sync.dma_start(out=bvSb, in_=bv_h[:, :])
            nc.sync.dma_start(out=xsb[:, 0:4, 0:TOK_T], in_=xin[:, 0:4, 0:TOK_T])
            nc.sync.dma_start(out=Wsb[:, :, 128:256], in_=Win[:, :, 128:256])
            nc.sync.dma_start(out=xsb[:, 4:8, 0:TOK_T], in_=xin[:, 4:8, 0:TOK_T])
            nc.sync.dma_start(out=Wsb[:, :, 0:128], in_=Win[:, :, 0:128])
            dma_x(1)
            nc.sync.dma_start(out=Wsb[:, :, 256:384], in_=Win[:, :, 256:384])
            for t in range(2, 4):
                dma_x(t)
            nc.sync.dma_start(out=WoSb, in_=Wo_h[:, :])
            for t in range(4, 8):
                dma_x(t)
            # PE p-state warm-up: harmless matmuls on a zeroed tile keep the
            # tensor engine ramping to full clock while the first DMAs land
            warm = consts.tile([128, TOK_T], bf16, tag="warm")
            nc.vector.memset(warm, 0.0)
            for w in range(12):
                pw = ps_o.tile([KB, TOK_T], f32, tag="po0", name="pw")
                nc.tensor.matmul(pw, warm[:, 0:128], warm,
                                 start=True, stop=True)
            make_identity(nc, ident)
            nc.vector.memset(onesrow, 1.0)
            nc.vector.memset(eps128, EPS)
            nc.vector.memset(zb128, 0.0)
            nc.vector.memset(ones2, 0.0)
            nc.vector.memset(ones2[0:64, 0:1], 1.0)
            nc.vector.memset(ones2[64:128, 1:2], 1.0)
            # vN ones + pad columns (softmax denominator trick)
            nc.vector.memset(vN[:, :, HD:HD + 1], 1.0)
            nc.vector.memset(vN[:, :, HD + 1:HD + 2], 0.0)
            nc.vector.memset(vN[:, :, VW + HD:VW + HD + 1], 1.0)
            nc.vector.memset(vN[:, :, VW + HD + 1:VW + HD + 2], 0.0)

            # ---------------- phase-1 pieces ----------------
            tile_state = {}

            def p_qkproj(t, m):
                # q (m=0) or k (m=1) projection; cast+bias; square
                tsl = slice(t * TOK_T, (t + 1) * TOK_T)
                px = ps_x.tile([128, TOK_T], f32, tag="px", name="pxqk")
                for kc in range(n_kc):
                    nc.tensor.matmul(
                        px,
                        Wsb[:, kc, m * 128:(m + 1) * 128],
                        xsb[:, kc, tsl],
                        start=(kc == 0),
                        stop=(kc == n_kc - 1),
                    )
                raw = p1s.tile([128, TOK_T], bf16, tag=f"raw{m}",
                               name=f"raw{m}")
                nc.vector.tensor_scalar_add(raw, px, bqSb[:, m:m + 1])
                st = tile_state.setdefault(t, {})
                if "sq" not in st:
                    st["sq"] = p1s.tile([128, 2, TOK_T], bf16, tag="sq",
                                        name="sq", bufs=2)
                nc.vector.tensor_mul(st["sq"][:, m, :], raw, raw)
                st[f"raw{m}"] = raw

            def p_norm(t):
                # transposed mean-square stats + rstd = exp(-0.5 ln(ms+eps))
                st = tile_state[t]
                sq = st["sq"]
                msx = ps_x.tile([128, TOK_T], f32, tag="px", name="msx")
                msv = msx[:, 0:16].rearrange("p (c m h) -> p c m h", c=4, m=2)
                for c in range(4):
                    for m in range(2):
                        nc.tensor.matmul(
                            msv[:, c, m, :],
                            sq[:, m, c * 128:(c + 1) * 128],
                            ones2,
                            start=True, stop=True)
                lnv = p1s.tile([128, 16], f32, tag="lnv", name="lnv")
                nc.scalar.activation(out=lnv, in_=msx[:, 0:16], func=AF.Ln,
                                     bias=eps128[:, :], scale=1.0 / HD)
                rstdT = p1s.tile([128, 4, 2, 2], bf16, tag="rstdT",
                                 name="rstdT")
                nc.scalar.activation(
                    out=rstdT.rearrange("p c m h -> p (c m h)"), in_=lnv,
                    func=AF.Exp, bias=zb128[:, :], scale=-0.5)
                st["rstdT"] = rstdT

            def p_qkout(t, m):
                # broadcast rstd over head dims: materialize [tok, hd] rows
                # on the DVE (stride-0 read), then transpose via identity
                # matmul; normalize on the DVE
                st = tile_state[t]
                tsl = slice(t * TOK_T, (t + 1) * TOK_T)
                bcT = p1s.tile([128, 4, 128], bf16, tag="bcT", name="bcT")
                nc.vector.tensor_copy(
                    bcT.rearrange("p c (h x) -> p c h x", h=2),
                    st["rstdT"][:, :, m, :].unsqueeze(3).broadcast_to(
                        [128, 4, 2, HD]))
                bc = ps_x.tile([128, TOK_T], f32, tag="px", name="bc")
                for c in range(4):
                    nc.tensor.matmul(bc[:, c * 128:(c + 1) * 128],
                                     bcT[:, c, :], ident,
                                     start=True, stop=True)
                if m == 0:
                    nc.vector.scalar_tensor_tensor(
                        out=qnT[:, tsl], in0=st["raw0"],
                        scalar=qksSb[:, 0:1], in1=bc,
                        op0=ALU.mult, op1=ALU.mult)
                else:
                    nc.vector.tensor_mul(knT[:, tsl], st["raw1"], bc)

            def p_v(t):
                # v in natural [token, hd] layout (x stationary)
                pv = ps_x.tile([128, 4, 128], f32, tag="px", name="pv")
                for j in range(4):
                    tok0 = t * TOK_T + j * 128
                    for kc in range(n_kc):
                        nc.tensor.matmul(
                            pv[:, j, :],
                            xsb[:, kc, tok0:tok0 + 128],
                            Wsb[:, kc, 256:384],
                            start=(kc == 0),
                            stop=(kc == n_kc - 1) and not has_qkv_bias,
                        )
                    if has_qkv_bias:
                        nc.tensor.matmul(pv[:, j, :], onesrow[:, 0:128], bvSb,
                                         start=False, stop=True)
                c0 = t * 4
                dst = vN[:, c0:c0 + 4, :].rearrange(
                    "p c (h x) -> p c h x", h=2)[:, :, :, 0:HD]
                src = pv.rearrange("p c (h x) -> p c h x", h=2)
                nc.vector.tensor_copy(dst, src)

            # ---------------- work queue ----------------
            work = []
            pending_labels = set()

            def push(cost, fn, label=None):
                if label is not None:
                    pending_labels.add(label)
                work.append((cost, fn, label))

            def _pop_one():
                c, fn, label = work.pop(0)
                fn()
                if label is not None:
                    pending_labels.discard(label)
                return c

            def feed(budget):
                spent = 0
                while work and spent < budget:
                    spent += _pop_one()

            def drain_through(label):
                # emit queued pieces until `label` has been emitted; no-op if
                # the label was never pushed or already done
                while label in pending_labels:
                    _pop_one()

            def push_chain(t):
                # order chosen so PE-independent work (v proj) separates the
                # chain's cross-engine dependencies (cast/sq -> msq -> rstd)
                push(1800, lambda: p_qkproj(t, 1))
                push(1800, lambda: p_qkproj(t, 0))
                push(700, lambda: p_norm(t))
                push(700, lambda: p_qkout(t, 1), ("kout", t))
                push(700, lambda: p_qkout(t, 0), ("qout", t))
                push(2000, lambda: p_v(t), ("v", t))

            def push_chain_pair(a, b):
                # interleave two tiles' chains: tile b's projections hide
                # tile a's cross-engine (DVE/ACT) normalization latency
                push(1800, lambda: p_qkproj(a, 1))
                push(1800, lambda: p_qkproj(a, 0))
                push(1800, lambda: p_qkproj(b, 1))
                push(700, lambda: p_norm(a))
                push(1800, lambda: p_qkproj(b, 0))
                push(700, lambda: p_qkout(a, 1), ("kout", a))
                push(700, lambda: p_norm(b))
                push(700, lambda: p_qkout(a, 0), ("qout", a))
                push(700, lambda: p_qkout(b, 1), ("kout", b))
                push(2000, lambda: p_v(a), ("v", a))
                push(700, lambda: p_qkout(b, 0), ("qout", b))
                push(2000, lambda: p_v(b), ("v", b))

            # ---------------- attention unit ----------------
            uid_ctr = [0]

            def emit_unit(b, qt, last=False, next_q=None, defer_ops=False):
                uid = uid_ctr[0]
                uid_ctr[0] += 1
                q0 = b * S + qt * TOK_T
                qsl = slice(q0, q0 + TOK_T)
                drain_through(("qout", b * 4 + qt))
                onT = onTp.tile([PD, TOK_T], bf16, tag="onT", name="onT")
                po = {}
                pov = {}
                for h in range(HPC):
                    po[h] = ps_o.tile([KB, TOK_T], f32, tag=f"po{h}",
                                      name=f"po{h}")
                    pov[h] = po[h][:, 0:4 * VW].rearrange(
                        "p (a b) -> p a b", a=4)

                def finish_head(h, is_last_head=False):
                    def norm():
                        rec = recp.tile([128, 4, 1], f32, tag="rec",
                                        name="rec")
                        nc.vector.reciprocal(rec, pov[h][:, :, HD:HD + 1])
                        if ("on2", uid) not in tile_state:
                            tile_state[("on2", uid)] = onp.tile(
                                [128, 4, HPC, HD], bf16, tag="on2",
                                name="on2")
                        o_n2 = tile_state[("on2", uid)]
                        nc.vector.tensor_mul(
                            o_n2[:, :, h, :], pov[h][:, :, 0:HD],
                            rec.broadcast_to([128, 4, HD]))

                    def transp():
                        # both heads adjacent in free dim -> one 128x128
                        # xbar-tile DMA transpose per query sub-block
                        o_n2 = tile_state.pop(("on2", uid))
                        for qs in range(4):
                            nc.sync.dma_start_transpose(
                                out=onT[:, qs * 128:(qs + 1) * 128],
                                in_=o_n2[:, qs, :, :])

                    push(500, norm, ("norm", uid, h))
                    if is_last_head:
                        push(300, transp)

                def pv_chunk(pts_h, h, qs):
                    # one full accumulation group (all 16 kb) per qs slice:
                    # only one pending PSUM group per bank is legal
                    for kb in range(n_kb):
                        g, j = kb // NG, kb % NG
                        nc.tensor.matmul(
                            pov[h][:, qs, 0:HD + 1],
                            pts_h[g][:, j, qs * 128:(qs + 1) * 128],
                            vN[:, b * n_kb + kb, h * VW:h * VW + HD + 1],
                            start=(kb == 0),
                            stop=(kb == n_kb - 1),
                        )

                h_order = (1, 0) if last else (0, 1)
                pts = {h: [] for h in h_order}
                prev_h = None
                for h in h_order:
                    for g in range(n_kb // NG):
                        drain_through(("kout", b * 4 + g // 2))
                        hsl = slice(h * HD, (h + 1) * HD)
                        pss = ps4.tile([KB, NG, TOK_T], f32, tag="pss",
                                       name="pss")
                        for j in range(NG):
                            kb = g * NG + j
                            k0 = b * S + kb * KB
                            nc.tensor.matmul(pss[:, j, :],
                                             knT[hsl, k0:k0 + KB],
                                             qnT[hsl, qsl],
                                             start=True, stop=True)
                        pt = ptp.tile([KB, NG, TOK_T], bf16, tag="pt",
                                      name="pt")
                        nc.scalar.activation(out=pt, in_=pss, func=AF.Exp,
                                             bias=zb128[:, :], scale=1.0)
                        pts[h].append(pt)
                        if prev_h is not None and g % 2 == 1:
                            if g == 1:
                                drain_through(("v", b * 4 + 3))
                                if uid > 0:
                                    drain_through(("norm", uid - 1, prev_h))
                            pv_chunk(pts[prev_h], prev_h, g // 2)
                            if g == n_kb // NG - 1:
                                finish_head(prev_h)
                        if prev_h is not None and g == 4 and next_q is not None:
                            drain_through(next_q)
                        if prev_h is not None and g == 6:
                            drain_through(("kout", (b * 4 + 4) % 8))
                        feed(1100)
                    prev_h = h
                if uid > 0:
                    drain_through(("norm", uid - 1, prev_h))
                for qs in range(4):
                    push(600, (lambda qs=qs: pv_chunk(pts[prev_h], prev_h,
                                                      qs)))
                finish_head(prev_h, is_last_head=True)

                ot_ref = {}
                op_pieces = []
                for tb in range(4):
                    for od in range(2):
                        def oph(tb=tb, od=od):
                            if od == 0:
                                ot_ref[tb] = otp.tile([128, D], bf16,
                                                      tag="ot", name="ot")
                            ot = ot_ref[tb]
                            px = ps_x.tile([128, TOK_T], f32, tag="px",
                                           name="pxop")
                            nc.tensor.matmul(
                                px,
                                onT[:, tb * 128:(tb + 1) * 128],
                                WoSb[:, od * TOK_T:(od + 1) * TOK_T],
                                start=True, stop=True)
                            if last and od == 1:
                                nc.scalar.copy(
                                    out=ot[:, od * TOK_T:(od + 1) * TOK_T],
                                    in_=px)
                            else:
                                nc.vector.tensor_copy(
                                    ot[:, od * TOK_T:(od + 1) * TOK_T], px)
                            if od == 1:
                                r0 = q0 + tb * 128
                                nc.sync.dma_start(
                                    out=out_h[r0:r0 + 128, :], in_=ot)
                        op_pieces.append(oph)
                if defer_ops:
                    return op_pieces
                for p in op_pieces:
                    push(600, p)

            # ---------------- emission schedule ----------------
            # head: interleaved chains for tiles 0-1 (v deferred to feeds)
            p_qkproj(0, 1)
            p_qkproj(0, 0)
            p_qkproj(1, 1)
            p_norm(0)
            p_qkproj(1, 0)
            p_qkout(0, 1)
            p_norm(1)
            p_qkout(0, 0)
            p_qkout(1, 1)
            p_qkout(1, 0)
            push(2000, lambda: p_v(0), ("v", 0))
            push(2000, lambda: p_v(1), ("v", 1))
            push_chain_pair(2, 3)
            dops = []
            dops.append(emit_unit(0, 0, next_q=("qout", 1), defer_ops=True))
            push_chain_pair(4, 5)
            dops.append(emit_unit(0, 1, next_q=("qout", 2), defer_ops=True))
            dops.append(emit_unit(0, 2, next_q=("qout", 3), defer_ops=True))
            push_chain_pair(6, 7)
            dops.append(emit_unit(0, 3, next_q=("qout", 4), defer_ops=True))
            for p in dops[0]:
                push(600, p)
            emit_unit(1, 0, next_q=("qout", 5))
            for p in dops[1]:
                push(600, p)
            emit_unit(1, 1, next_q=("qout", 6))
            for p in dops[2]:
                push(600, p)
            emit_unit(1, 2, next_q=("qout", 7))
            for p in dops[3]:
                push(600, p)
            emit_unit(1, 3, last=True)
            feed(10 ** 9)  # drain

    nc.compile()
    return nc


def kernel(x, Wqkv, bqkv, Wout, bout, q_scale, k_scale):
    global _last_results
    import ml_dtypes
    from concourse.bass_utils import run_bass_kernel_spmd

    bf16 = ml_dtypes.bfloat16
    x = np.asarray(x, dtype=np.float32)
    Wqkv = np.asarray(Wqkv, dtype=np.float32)
    bqkv = np.asarray(bqkv, dtype=np.float32)
    Wout = np.asarray(Wout, dtype=np.float32)
    bout = np.asarray(bout, dtype=np.float32)
    q_scale = np.asarray(q_scale, dtype=np.float32)
    k_scale = np.asarray(k_scale, dtype=np.float32)

    has_qkv_bias = bool(np.any(bqkv != 0.0))

    xT = np.ascontiguousarray(x.reshape(N, D).T).astype(bf16)
    # k_scale folds into the q side (scores are bilinear in q, k)
    qks = (np.tile(q_scale * k_scale, HPC).reshape(PD, 1)
           / np.sqrt(HD)).astype(np.float32)

    in_maps = []
    for c in range(NCORES):
        c0 = c * PD
        Wq_s = np.concatenate(
            [Wqkv[:, c0:c0 + PD], Wqkv[:, D + c0:D + c0 + PD],
             Wqkv[:, 2 * D + c0:2 * D + c0 + PD]], axis=1).astype(bf16)
        bq_s = np.ascontiguousarray(np.stack(
            [bqkv[c0:c0 + PD], bqkv[D + c0:D + c0 + PD],
             bqkv[2 * D + c0:2 * D + c0 + PD]], axis=1))
        bv_s = bqkv[2 * D + c0:2 * D + c0 + PD].reshape(1, PD).astype(bf16)
        Wo_s = np.ascontiguousarray(Wout[c0:c0 + PD, :]).astype(bf16)
        in_maps.append({"xT": xT, "Wq": Wq_s, "bq": bq_s, "bv": bv_s,
                        "Wo": Wo_s, "qks": qks})

    nc = _build_program(has_qkv_bias)
    res = run_bass_kernel_spmd(nc, in_maps, core_ids=list(range(NCORES)))
    _last_results = res

    acc = res.results[0]["outp"].astype(np.float32)
    for c in range(1, NCORES):
        acc = acc + res.results[c]["outp"].astype(np.float32)
    acc = acc + bout
    return acc.reshape(B, S, D).astype(np.float32)


# revision 38
# speedup vs baseline: 1.2465x; 1.0063x over previous
"""Fused multi-head attention block (qkv proj + RMSNorm(q,k) + softmax(QK^T)V
+ out proj), tensor-parallel over 8 TRN2 NeuronCores (2 heads per core).

Cost-model-driven design:
  - All matmul operands bf16 (1 cycle/row at any free size).
  - PV computed transposed: o[q, hd] += P[k, q]^T v[k, hd] with V as the
    small moving operand (F=66 incl. a ones-column that accumulates the
    softmax denominator) -> ~2x cheaper than the natural orientation, and
    the denominator becomes a per-partition scalar for the DVE.
  - v projected directly into natural [token, hd] layout (x stationary).
  - k_scale folded into the q-side scale on the host (scores are bilinear).
  - RMS statistics computed transposed ([token, head] via tiny F=2 matmuls)
    so rstd = exp(-0.5*ln(msq+eps)) runs on 16-element ACT ops; the whole
    kernel then uses ONE ACT table (Ln+Exp) -> no table reloads. rstd is
    broadcast back over head dims by an identity matmul with a stride-0
    weights view.
  - Scores accumulate in double-buffered 2-bank PSUM tiles; exp() runs on
    [128, 1024] elements per instruction. PSUM budget (8 banks): score
    tiles 2x2 + two PV accumulators + two rotating scratch banks.
  - Only one pending PSUM accumulation group per 2KB bank is legal on HW
    (start=True resets the bank's has_written bits), so each PV qs-slice
    runs as one complete 16-block group; exp outputs are buffered deep.
  - A labeled work-queue feeds phase-1 (projections/norm) and out-proj
    pieces into the attention unit loops in ~1us chunks with explicit
    producer->consumer drain barriers, so the in-order engine queues never
    head-block and the PE stays dense. Paired tiles' chains interleave so
    one tile's projections hide the other's cross-engine norm latency;
    batch-0 out-projections are deferred into the ACT-bound batch-1 units.
  - o[q, hd] transposed back on the PE (bf16, 1 c/r) for the out proj.
  - Per-core partial outputs written bf16; host sums in fp32 (+bout).
"""

import numpy as np

B, S, D, H = 2, 2048, 1024, 16
HD = D // H            # 64
N = B * S              # 4096 tokens
NCORES = 8
HPC = H // NCORES      # 2 heads per core
PD = HPC * HD          # 128 per-core head dims
EPS = 1e-6
TOK_T = 512            # token tile / query tile
KB = 128               # key block (contraction dim of PV)
VW = HD + 2            # per-head v width (64 dims + ones col + pad)

_last_results = None   # test.py introspection
_nc_cache = {}
_last_flag = [False]


def _build_program(has_qkv_bias=None):
    if has_qkv_bias is None:
        has_qkv_bias = _last_flag[0]
    if has_qkv_bias not in _nc_cache:
        _nc_cache[has_qkv_bias] = _build_program_uncached(has_qkv_bias)
    _last_flag[0] = has_qkv_bias
    return _nc_cache[has_qkv_bias]


def _build_program_uncached(has_qkv_bias):
    import concourse.bacc as bacc
    import concourse.bass as bass
    import concourse.mybir as mybir
    import concourse.tile as tile
    from concourse.masks import make_identity

    f32 = mybir.dt.float32
    bf16 = mybir.dt.bfloat16
    AF = mybir.ActivationFunctionType
    ALU = mybir.AluOpType

    nc = bacc.Bacc(None, target_bir_lowering=False, debug=False)

    xT_h = nc.declare_dram_parameter("xT", [D, N], bf16, isOutput=False)
    Wq_h = nc.declare_dram_parameter("Wq", [D, 3 * PD], bf16, isOutput=False)
    bq_h = nc.declare_dram_parameter("bq", [PD, 3], f32, isOutput=False)
    bv_h = nc.declare_dram_parameter("bv", [1, PD], bf16, isOutput=False)
    Wo_h = nc.declare_dram_parameter("Wo", [PD, D], bf16, isOutput=False)
    qks_h = nc.declare_dram_parameter("qks", [PD, 1], f32, isOutput=False)
    out_h = nc.declare_dram_parameter("outp", [N, D], bf16, isOutput=True)

    n_kc = D // 128             # 8 contraction chunks for qkv proj
    n_kb = S // KB              # 16 key blocks per batch
    NG = 2                      # kb per score group (2-bank psum tile, x2 bufs)

    # Force one ACT function table (Ln+Exp) for the whole program: pass the
    # table list with every other set emptied (indices preserved so
    # act_func_set_id still points into act_info.json's act_func_sets).
    import types as _types
    from concourse.hw_specs import get_activation_tables as _gat

    def _one_table_loads(self):
        import bass_rust as _bass_rust
        has_activation = any(
            isinstance(i, mybir.InstActivation)
            for b in self.main_func.blocks
            for i in b.instructions
        )
        if not has_activation:
            return
        tabs = [
            (name, (funcs if name == "natural_log_exp_and_others" else set()))
            for name, funcs in _gat(self.m.arch).items()
        ]
        _bass_rust.insert_act_table_loads(self, tabs)

    nc.insert_act_table_loads = _types.MethodType(_one_table_loads, nc)

    with nc.allow_low_precision(reason="bf16 matmul pipeline"), \
            tile.TileContext(nc) as tc:
        with (
            tc.tile_pool(name="big", bufs=1) as big,
            tc.tile_pool(name="consts", bufs=1) as consts,
            tc.tile_pool(name="p1s", bufs=3) as p1s,
            tc.tile_pool(name="ptp", bufs=20) as ptp,
            tc.tile_pool(name="onp", bufs=2) as onp,
            tc.tile_pool(name="onTp", bufs=6) as onTp,
            tc.tile_pool(name="otp", bufs=4) as otp,
            tc.tile_pool(name="recp", bufs=2) as recp,
            tc.tile_pool(name="ps4", bufs=2, space=bass.MemorySpace.PSUM) as ps4,
            tc.tile_pool(name="ps_o", bufs=1, space=bass.MemorySpace.PSUM) as ps_o,
            tc.tile_pool(name="ps_x", bufs=2, space=bass.MemorySpace.PSUM) as ps_x,
        ):
            # ---- persistent SBUF tensors ----
            xsb = big.tile([128, n_kc, N], bf16, tag="xsb")
            qnT = big.tile([PD, N], bf16, tag="qnT")
            knT = big.tile([PD, N], bf16, tag="knT")
            vN = big.tile([KB, N // KB, 2 * VW], bf16, tag="vN")
            Wsb = big.tile([128, n_kc, 3 * PD], bf16, tag="Wsb")
            WoSb = big.tile([PD, D], bf16, tag="WoSb")

            bqSb = consts.tile([PD, 3], f32, tag="bqSb")
            bvSb = consts.tile([1, PD], bf16, tag="bvSb")
            qksSb = consts.tile([PD, 1], f32, tag="qksSb")
            ones2 = consts.tile([128, 2], bf16, tag="ones2")
            ident = consts.tile([128, 128], bf16, tag="ident")
            onesrow = consts.tile([1, KB], bf16, tag="onesrow")
            eps128 = consts.tile([128, 1], f32, tag="eps128")
            zb128 = consts.tile([128, 1], f32, tag="zb128")

            Win = Wq_h[:, :].rearrange("(kc p) j -> p kc j", p=128)
            xin = xT_h[:, :].rearrange("(kc p) n -> p kc n", p=128)

            # early DMAs, ordered so tile-0 chains can start ASAP
            def dma_x(t):
                tsl = slice(t * TOK_T, (t + 1) * TOK_T)
                nc.sync.dma_start(out=xsb[:, 0:4, tsl], in_=xin[:, 0:4, tsl])
                nc.sync.dma_start(out=xsb[:, 4:8, tsl], in_=xin[:, 4:8, tsl])

            nc.sync.dma_start(out=bqSb, in_=bq_h[:, :])
            nc.sync.dma_start(out=qksSb, in_=qks_h[:, :])
            nc.sync.dma_start(out=bvSb, in_=bv_h[:, :])
            nc.sync.dma_start(out=xsb[:, 0:4, 0:TOK_T], in_=xin[:, 0:4, 0:TOK_T])
            nc.sync.dma_start(out=Wsb[:, :, 128:256], in_=Win[:, :, 128:256])
            nc.sync.dma_start(out=xsb[:, 4:8, 0:TOK_T], in_=xin[:, 4:8, 0:TOK_T])
            nc.sync.dma_start(out=Wsb[:, :, 0:128], in_=Win[:, :, 0:128])
            dma_x(1)
            nc.sync.dma_start(out=Wsb[:, :, 256:384], in_=Win[:, :, 256:384])
            for t in range(2, 4):
                dma_x(t)
            nc.sync.dma_start(out=WoSb, in_=Wo_h[:, :])
            for t in range(4, 8):
                dma_x(t)
            # PE p-state warm-up: harmless matmuls on a zeroed tile keep the
            # tensor engine ramping to full clock while the first DMAs land
            warm = consts.tile([128, TOK_T], bf16, tag="warm")
            nc.vector.memset(warm, 0.0)
            for w in range(12):
                pw = ps_o.tile([KB, TOK_T], f32, tag="po0", name="pw")
                nc.tensor.matmul(pw, warm[:, 0:128], warm,
                                 start=True, stop=True)
            make_identity(nc, ident)
            nc.vector.memset(onesrow, 1.0)
            nc.vector.memset(eps128, EPS)
            nc.vector.memset(zb128, 0.0)
            nc.vector.memset(ones2, 0.0)
            nc.vector.memset(ones2[0:64, 0:1], 1.0)
            nc.vector.memset(ones2[64:128, 1:2], 1.0)
            # vN ones + pad columns (softmax denominator trick)
            nc.vector.memset(vN[:, :, HD:HD + 1], 1.0)
            nc.vector.memset(vN[:, :, HD + 1:HD + 2], 0.0)
            nc.vector.memset(vN[:, :, VW + HD:VW + HD + 1], 1.0)
            nc.vector.memset(vN[:, :, VW + HD + 1:VW + HD + 2], 0.0)

            # ---------------- phase-1 pieces ----------------
            tile_state = {}

            def p_qkproj(t, m):
                # q (m=0) or k (m=1) projection; cast+bias; square
                tsl = slice(t * TOK_T, (t + 1) * TOK_T)
                px = ps_x.tile([128, TOK_T], f32, tag="px", name="pxqk")
                for kc in range(n_kc):
                    nc.tensor.matmul(
                        px,
                        Wsb[:, kc, m * 128:(m + 1) * 128],
                        xsb[:, kc, tsl],
                        start=(kc == 0),
                        stop=(kc == n_kc - 1),
                    )
                raw = p1s.tile([128, TOK_T], bf16, tag=f"raw{m}",
                               name=f"raw{m}")
                nc.vector.tensor_scalar_add(raw, px, bqSb[:, m:m + 1])
                st = tile_state.setdefault(t, {})
                if "sq" not in st:
                    st["sq"] = p1s.tile([128, 2, TOK_T], bf16, tag="sq",
                                        name="sq", bufs=2)
                nc.vector.tensor_mul(st["sq"][:, m, :], raw, raw)
                st[f"raw{m}"] = raw

            def p_norm(t):
                # transposed mean-square stats + rstd = exp(-0.5 ln(ms+eps))
                st = tile_state[t]
                sq = st["sq"]
                msx = ps_x.tile([128, TOK_T], f32, tag="px", name="msx")
                msv = msx[:, 0:16].rearrange("p (c m h) -> p c m h", c=4, m=2)
                for c in range(4):
                    for m in range(2):
                        nc.tensor.matmul(
                            msv[:, c, m, :],
                            sq[:, m, c * 128:(c + 1) * 128],
                            ones2,
                            start=True, stop=True)
                lnv = p1s.tile([128, 16], f32, tag="lnv", name="lnv")
                nc.scalar.activation(out=lnv, in_=msx[:, 0:16], func=AF.Ln,
                                     bias=eps128[:, :], scale=1.0 / HD)
                rstdT = p1s.tile([128, 4, 2, 2], bf16, tag="rstdT",
                                 name="rstdT")
                nc.scalar.activation(
                    out=rstdT.rearrange("p c m h -> p (c m h)"), in_=lnv,
                    func=AF.Exp, bias=zb128[:, :], scale=-0.5)
                st["rstdT"] = rstdT

            def p_qkout(t, m):
                # broadcast rstd over head dims: materialize [tok, hd] rows
                # on the DVE (stride-0 read), then transpose via identity
                # matmul; normalize on the DVE
                st = tile_state[t]
                tsl = slice(t * TOK_T, (t + 1) * TOK_T)
                bcT = p1s.tile([128, 4, 128], bf16, tag="bcT", name="bcT")
                nc.vector.tensor_copy(
                    bcT.rearrange("p c (h x) -> p c h x", h=2),
                    st["rstdT"][:, :, m, :].unsqueeze(3).broadcast_to(
                        [128, 4, 2, HD]))
                bc = ps_x.tile([128, TOK_T], f32, tag="px", name="bc")
                for c in range(4):
                    nc.tensor.matmul(bc[:, c * 128:(c + 1) * 128],
                                     bcT[:, c, :], ident,
                                     start=True, stop=True)
                if m == 0:
                    nc.vector.scalar_tensor_tensor(
                        out=qnT[:, tsl], in0=st["raw0"],
                        scalar=qksSb[:, 0:1], in1=bc,
                        op0=ALU.mult, op1=ALU.mult)
                else:
                    nc.vector.tensor_mul(knT[:, tsl], st["raw1"], bc)

            def p_v(t):
                # v in natural [token, hd] layout (x stationary)
                pv = ps_x.tile([128, 4, 128], f32, tag="px", name="pv")
                for j in range(4):
                    tok0 = t * TOK_T + j * 128
                    for kc in range(n_kc):
                        nc.tensor.matmul(
                            pv[:, j, :],
                            xsb[:, kc, tok0:tok0 + 128],
                            Wsb[:, kc, 256:384],
                            start=(kc == 0),
                            stop=(kc == n_kc - 1) and not has_qkv_bias,
                        )
                    if has_qkv_bias:
                        nc.tensor.matmul(pv[:, j, :], onesrow[:, 0:128], bvSb,
                                         start=False, stop=True)
                c0 = t * 4
                dst = vN[:, c0:c0 + 4, :].rearrange(
                    "p c (h x) -> p c h x", h=2)[:, :, :, 0:HD]
                src = pv.rearrange("p c (h x) -> p c h x", h=2)
                nc.vector.tensor_copy(dst, src)

            # ---------------- work queue ----------------
            work = []
            pending_labels = set()

            def push(cost, fn, label=None):
                if label is not None:
                    pending_labels.add(label)
                work.append((cost, fn, label))

            def _pop_one():
                c, fn, label = work.pop(0)
                fn()
                if label is not None:
                    pending_labels.discard(label)
                return c

            def feed(budget):
                spent = 0
                while work and spent < budget:
                    spent += _pop_one()

            def drain_through(label):
                # emit queued pieces until `label` has been emitted; no-op if
                # the label was never pushed or already done
                while label in pending_labels:
                    _pop_one()

            def push_chain(t):
                # order chosen so PE-independent work (v proj) separates the
                # chain's cross-engine dependencies (cast/sq -> msq -> rstd)
                push(1800, lambda: p_qkproj(t, 1))
                push(1800, lambda: p_qkproj(t, 0))
                push(700, lambda: p_norm(t))
                push(700, lambda: p_qkout(t, 1), ("kout", t))
                push(700, lambda: p_qkout(t, 0), ("qout", t))
                push(2000, lambda: p_v(t), ("v", t))

            def push_chain_pair(a, b):
                # interleave two tiles' chains: tile b's projections hide
                # tile a's cross-engine (DVE/ACT) normalization latency
                push(1800, lambda: p_qkproj(a, 1))
                push(1800, lambda: p_qkproj(a, 0))
                push(1800, lambda: p_qkproj(b, 1))
                push(700, lambda: p_norm(a))
                push(1800, lambda: p_qkproj(b, 0))
                push(700, lambda: p_qkout(a, 1), ("kout", a))
                push(700, lambda: p_norm(b))
                push(700, lambda: p_qkout(a, 0), ("qout", a))
                push(700, lambda: p_qkout(b, 1), ("kout", b))
                push(2000, lambda: p_v(a), ("v", a))
                push(700, lambda: p_qkout(b, 0), ("qout", b))
                push(2000, lambda: p_v(b), ("v", b))

            # ---------------- attention unit ----------------
            uid_ctr = [0]

            def emit_unit(b, qt, last=False, next_q=None, defer_ops=False):
                uid = uid_ctr[0]
                uid_ctr[0] += 1
                q0 = b * S + qt * TOK_T
                qsl = slice(q0, q0 + TOK_T)
                drain_through(("qout", b * 4 + qt))
                onT = onTp.tile([PD, TOK_T], bf16, tag="onT", name="onT")
                po = {}
                pov = {}
                for h in range(HPC):
                    po[h] = ps_o.tile([KB, TOK_T], f32, tag=f"po{h}",
                                      name=f"po{h}")
                    pov[h] = po[h][:, 0:4 * VW].rearrange(
                        "p (a b) -> p a b", a=4)

                def finish_head(h, is_last_head=False):
                    def norm():
                        rec = recp.tile([128, 4, 1], f32, tag="rec",
                                        name="rec")
                        nc.vector.reciprocal(rec, pov[h][:, :, HD:HD + 1])
                        if ("on2", uid) not in tile_state:
                            tile_state[("on2", uid)] = onp.tile(
                                [128, 4, HPC, HD], bf16, tag="on2",
                                name="on2")
                        o_n2 = tile_state[("on2", uid)]
                        nc.vector.tensor_mul(
                            o_n2[:, :, h, :], pov[h][:, :, 0:HD],
                            rec.broadcast_to([128, 4, HD]))

                    def transp():
                        # both heads adjacent in free dim -> one 128x128
                        # xbar-tile DMA transpose per query sub-block; the
                        # last unit uses PE transposes instead (lower
                        # latency -- nothing hides the DMA turnaround there)
                        o_n2 = tile_state.pop(("on2", uid))
                        if last:
                            tp = ps_x.tile([PD, 4, 128], bf16, tag="px",
                                           name="tp")
                            for qs in range(4):
                                nc.tensor.transpose(
                                    tp[:, qs, :],
                                    o_n2[:, qs, :, :].rearrange(
                                        "p a b -> p (a b)"),
                                    ident)
                            nc.vector.tensor_copy(
                                onT[:, :].rearrange("p (a c) -> p a c", a=4),
                                tp)
                        else:
                            for qs in range(4):
                                nc.sync.dma_start_transpose(
                                    out=onT[:, qs * 128:(qs + 1) * 128],
                                    in_=o_n2[:, qs, :, :])

                    push(500, norm, ("norm", uid, h))
                    if is_last_head:
                        push(300, transp)

                def pv_chunk(pts_h, h, qs):
                    # one full accumulation group (all 16 kb) per qs slice:
                    # only one pending PSUM group per bank is legal
                    for kb in range(n_kb):
                        g, j = kb // NG, kb % NG
                        nc.tensor.matmul(
                            pov[h][:, qs, 0:HD + 1],
                            pts_h[g][:, j, qs * 128:(qs + 1) * 128],
                            vN[:, b * n_kb + kb, h * VW:h * VW + HD + 1],
                            start=(kb == 0),
                            stop=(kb == n_kb - 1),
                        )

                h_order = (1, 0) if last else (0, 1)
                pts = {h: [] for h in h_order}
                prev_h = None
                for h in h_order:
                    for g in range(n_kb // NG):
                        drain_through(("kout", b * 4 + g // 2))
                        hsl = slice(h * HD, (h + 1) * HD)
                        pss = ps4.tile([KB, NG, TOK_T], f32, tag="pss",
                                       name="pss")
                        for j in range(NG):
                            kb = g * NG + j
                            k0 = b * S + kb * KB
                            nc.tensor.matmul(pss[:, j, :],
                                             knT[hsl, k0:k0 + KB],
                                             qnT[hsl, qsl],
                                             start=True, stop=True)
                        pt = ptp.tile([KB, NG, TOK_T], bf16, tag="pt",
                                      name="pt")
                        nc.scalar.activation(out=pt, in_=pss, func=AF.Exp,
                                             bias=zb128[:, :], scale=1.0)
                        pts[h].append(pt)
                        if prev_h is not None and g % 2 == 1:
                            if g == 1:
                                drain_through(("v", b * 4 + 3))
                                if uid > 0:
                                    drain_through(("norm", uid - 1, prev_h))
                            pv_chunk(pts[prev_h], prev_h, g // 2)
                            if g == n_kb // NG - 1:
                                finish_head(prev_h)
                        if prev_h is not None and g == 4 and next_q is not None:
                            drain_through(next_q)
                        if prev_h is not None and g == 6:
                            drain_through(("kout", (b * 4 + 4) % 8))
                        feed(1100)
                    prev_h = h
                if uid > 0:
                    drain_through(("norm", uid - 1, prev_h))
                for qs in range(4):
                    push(600, (lambda qs=qs: pv_chunk(pts[prev_h], prev_h,
                                                      qs)))
                finish_head(prev_h, is_last_head=True)

                ot_ref = {}
                op_pieces = []
                for tb in range(4):
                    for od in range(2):
                        def oph(tb=tb, od=od):
                            if od == 0:
                                ot_ref[tb] = otp.tile([128, D], bf16,
                                                      tag="ot", name="ot")
                            ot = ot_ref[tb]
                            px = ps_x.tile([128, TOK_T], f32, tag="px",
                                           name="pxop")
                            nc.tensor.matmul(
                                px,
                                onT[:, tb * 128:(tb + 1) * 128],
                                WoSb[:, od * TOK_T:(od + 1) * TOK_T],
                                start=True, stop=True)
                            if last and od == 1:
                                nc.scalar.copy(
                                    out=ot[:, od * TOK_T:(od + 1) * TOK_T],
                                    in_=px)
                            else:
                                nc.vector.tensor_copy(
                                    ot[:, od * TOK_T:(od + 1) * TOK_T], px)
                            if od == 1:
                                r0 = q0 + tb * 128
                                nc.sync.dma_start(
                                    out=out_h[r0:r0 + 128, :], in_=ot)
                        op_pieces.append(oph)
                if defer_ops:
                    return op_pieces
                for p in op_pieces:
                    push(600, p)

            # ---------------- emission schedule ----------------
            # head: interleaved chains for tiles 0-1 (v deferred to feeds)
            p_qkproj(0, 1)
            p_qkproj(0, 0)
            p_qkproj(1, 1)
            p_norm(0)
            p_qkproj(1, 0)
            p_qkout(0, 1)
            p_norm(1)
            p_qkout(0, 0)
            p_qkout(1, 1)
            p_qkout(1, 0)
            push(2000, lambda: p_v(0), ("v", 0))
            push(2000, lambda: p_v(1), ("v", 1))
            push_chain_pair(2, 3)
            dops = []
            dops.append(emit_unit(0, 0, next_q=("qout", 1), defer_ops=True))
            push_chain_pair(4, 5)
            dops.append(emit_unit(0, 1, next_q=("qout", 2), defer_ops=True))
            dops.append(emit_unit(0, 2, next_q=("qout", 3), defer_ops=True))
            push_chain_pair(6, 7)
            dops.append(emit_unit(0, 3, next_q=("qout", 4), defer_ops=True))
            for p in dops[0]:
                push(600, p)
            emit_unit(1, 0, next_q=("qout", 5))
            for p in dops[1]:
                push(600, p)
            emit_unit(1, 1, next_q=("qout", 6))
            for p in dops[2]:
                push(600, p)
            emit_unit(1, 2, next_q=("qout", 7))
            for p in dops[3]:
                push(600, p)
            emit_unit(1, 3, last=True)
            feed(10 ** 9)  # drain

    nc.compile()
    return nc


def kernel(x, Wqkv, bqkv, Wout, bout, q_scale, k_scale):
    global _last_results
    import ml_dtypes
    from concourse.bass_utils import run_bass_kernel_spmd

    bf16 = ml_dtypes.bfloat16
    x = np.asarray(x, dtype=np.float32)
    Wqkv = np.asarray(Wqkv, dtype=np.float32)
    bqkv = np.asarray(bqkv, dtype=np.float32)
    Wout = np.asarray(Wout, dtype=np.float32)
    bout = np.asarray(bout, dtype=np.float32)
    q_scale = np.asarray(q_scale, dtype=np.float32)
    k_scale = np.asarray(k_scale, dtype=np.float32)

    has_qkv_bias = bool(np.any(bqkv != 0.0))

    xT = np.ascontiguousarray(x.reshape(N, D).T).astype(bf16)
    # k_scale folds into the q side (scores are bilinear in q, k)
    qks = (np.tile(q_scale * k_scale, HPC).reshape(PD, 1)
           / np.sqrt(HD)).astype(np.float32)

    in_maps = []
    for c in range(NCORES):
        c0 = c * PD
        Wq_s = np.concatenate(
            [Wqkv[:, c0:c0 + PD], Wqkv[:, D + c0:D + c0 + PD],
             Wqkv[:, 2 * D + c0:2 * D + c0 + PD]], axis=1).astype(bf16)
        bq_s = np.ascontiguousarray(np.stack(
            [bqkv[c0:c0 + PD], bqkv[D + c0:D + c0 + PD],
             bqkv[2 * D + c0:2 * D + c0 + PD]], axis=1))
        bv_s = bqkv[2 * D + c0:2 * D + c0 + PD].reshape(1, PD).astype(bf16)
        Wo_s = np.ascontiguousarray(Wout[c0:c0 + PD, :]).astype(bf16)
        in_maps.append({"xT": xT, "Wq": Wq_s, "bq": bq_s, "bv": bv_s,
                        "Wo": Wo_s, "qks": qks})

    nc = _build_program(has_qkv_bias)
    res = run_bass_kernel_spmd(nc, in_maps, core_ids=list(range(NCORES)))
    _last_results = res

    acc = res.results[0]["outp"].astype(np.float32)
    for c in range(1, NCORES):
        acc = acc + res.results[c]["outp"].astype(np.float32)
    acc = acc + bout
    return acc.reshape(B, S, D).astype(np.float32)


# revision 44
# speedup vs baseline: 1.2609x; 1.0116x over previous
"""Fused multi-head attention block (qkv proj + RMSNorm(q,k) + softmax(QK^T)V
+ out proj), tensor-parallel over 8 TRN2 NeuronCores (2 heads per core).

Cost-model-driven design:
  - All matmul operands bf16 (1 cycle/row at any free size).
  - PV computed transposed: o[q, hd] += P[k, q]^T v[k, hd] with V as the
    small moving operand (F=66 incl. a ones-column that accumulates the
    softmax denominator) -> ~2x cheaper than the natural orientation, and
    the denominator becomes a per-partition scalar for the DVE.
  - v projected directly into natural [token, hd] layout (x stationary).
  - k_scale folded into the q-side scale on the host (scores are bilinear).
  - RMS statistics computed transposed ([token, head] via tiny F=2 matmuls)
    so rstd = exp(-0.5*ln(msq+eps)) runs on 16-element ACT ops; the whole
    kernel then uses ONE ACT table (Ln+Exp) -> no table reloads. rstd is
    broadcast back over head dims by an identity matmul with a stride-0
    weights view.
  - Scores accumulate in double-buffered 2-bank PSUM tiles; exp() runs on
    [128, 1024] elements per instruction. PSUM budget (8 banks): score
    tiles 2x2 + two PV accumulators + two rotating scratch banks.
  - Only one pending PSUM accumulation group per 2KB bank is legal on HW
    (start=True resets the bank's has_written bits), so each PV qs-slice
    runs as one complete 16-block group; exp outputs are buffered deep.
  - A labeled work-queue feeds phase-1 (projections/norm) and out-proj
    pieces into the attention unit loops in ~1us chunks with explicit
    producer->consumer drain barriers, so the in-order engine queues never
    head-block and the PE stays dense. Paired tiles' chains interleave so
    one tile's projections hide the other's cross-engine norm latency;
    batch-0 out-projections are deferred into the ACT-bound batch-1 units.
  - o[q, hd] transposed back on the PE (bf16, 1 c/r) for the out proj.
  - Per-core partial outputs written bf16; host sums in fp32 (+bout).
"""

import numpy as np

B, S, D, H = 2, 2048, 1024, 16
HD = D // H            # 64
N = B * S              # 4096 tokens
NCORES = 8
HPC = H // NCORES      # 2 heads per core
PD = HPC * HD          # 128 per-core head dims
EPS = 1e-6
TOK_T = 512            # token tile / query tile
KB = 128               # key block (contraction dim of PV)
VW = HD + 2            # per-head v width (64 dims + ones col + pad)

_last_results = None   # test.py introspection
_nc_cache = {}
_last_flag = [False]


def _build_program(has_qkv_bias=None):
    if has_qkv_bias is None:
        has_qkv_bias = _last_flag[0]
    if has_qkv_bias not in _nc_cache:
        _nc_cache[has_qkv_bias] = _build_program_uncached(has_qkv_bias)
    _last_flag[0] = has_qkv_bias
    return _nc_cache[has_qkv_bias]


def _build_program_uncached(has_qkv_bias):
    n_kc0 = D // 128
    import concourse.bacc as bacc
    import concourse.bass as bass
    import concourse.mybir as mybir
    import concourse.tile as tile
    from concourse.masks import make_identity

    f32 = mybir.dt.float32
    bf16 = mybir.dt.bfloat16
    AF = mybir.ActivationFunctionType
    ALU = mybir.AluOpType

    nc = bacc.Bacc(None, target_bir_lowering=False, debug=False)

    xT_h = nc.declare_dram_parameter("xT", [D, N], bf16, isOutput=False)
    Wq_h = nc.declare_dram_parameter("Wq", [128, 3, n_kc0, 128], bf16,
                                 isOutput=False)
    bq_h = nc.declare_dram_parameter("bq", [PD, 3], f32, isOutput=False)
    bv_h = nc.declare_dram_parameter("bv", [1, PD], bf16, isOutput=False)
    Wo_h = nc.declare_dram_parameter("Wo", [PD, D], bf16, isOutput=False)
    qks_h = nc.declare_dram_parameter("qks", [PD, 1], f32, isOutput=False)
    out_h = nc.declare_dram_parameter("outp", [N, D], bf16, isOutput=True)

    n_kc = D // 128             # 8 contraction chunks for qkv proj
    n_kb = S // KB              # 16 key blocks per batch
    NG = 2                      # kb per score group (2-bank psum tile, x2 bufs)

    # Force one ACT function table (Ln+Exp) for the whole program: pass the
    # table list with every other set emptied (indices preserved so
    # act_func_set_id still points into act_info.json's act_func_sets).
    import types as _types
    from concourse.hw_specs import get_activation_tables as _gat

    def _one_table_loads(self):
        import bass_rust as _bass_rust
        has_activation = any(
            isinstance(i, mybir.InstActivation)
            for b in self.main_func.blocks
            for i in b.instructions
        )
        if not has_activation:
            return
        tabs = [
            (name, (funcs if name == "natural_log_exp_and_others" else set()))
            for name, funcs in _gat(self.m.arch).items()
        ]
        _bass_rust.insert_act_table_loads(self, tabs)

    nc.insert_act_table_loads = _types.MethodType(_one_table_loads, nc)

    with nc.allow_low_precision(reason="bf16 matmul pipeline"), \
            tile.TileContext(nc) as tc:
        with (
            tc.tile_pool(name="big", bufs=1) as big,
            tc.tile_pool(name="consts", bufs=1) as consts,
            tc.tile_pool(name="p1s", bufs=3) as p1s,
            tc.tile_pool(name="ptp", bufs=20) as ptp,
            tc.tile_pool(name="onp", bufs=2) as onp,
            tc.tile_pool(name="onTp", bufs=6) as onTp,
            tc.tile_pool(name="otp", bufs=4) as otp,
            tc.tile_pool(name="recp", bufs=2) as recp,
            tc.tile_pool(name="ps4", bufs=2, space=bass.MemorySpace.PSUM) as ps4,
            tc.tile_pool(name="ps_o", bufs=1, space=bass.MemorySpace.PSUM) as ps_o,
            tc.tile_pool(name="ps_x", bufs=2, space=bass.MemorySpace.PSUM) as ps_x,
        ):
            # ---- persistent SBUF tensors ----
            xsb = big.tile([128, n_kc, N], bf16, tag="xsb")
            qnT = big.tile([PD, N], bf16, tag="qnT")
            knT = big.tile([PD, N], bf16, tag="knT")
            vN = big.tile([KB, N // KB, 2 * VW], bf16, tag="vN")
            Wsb = big.tile([128, 3, n_kc, PD], bf16, tag="Wsb")
            WoSb = big.tile([PD, D], bf16, tag="WoSb")

            bqSb = consts.tile([PD, 3], f32, tag="bqSb")
            bvSb = consts.tile([1, PD], bf16, tag="bvSb")
            qksSb = consts.tile([PD, 1], f32, tag="qksSb")
            ones2 = consts.tile([128, 2], bf16, tag="ones2")
            ident = consts.tile([128, 128], bf16, tag="ident")
            onesrow = consts.tile([1, KB], bf16, tag="onesrow")
            eps128 = consts.tile([128, 1], f32, tag="eps128")
            zb128 = consts.tile([128, 1], f32, tag="zb128")

            xin = xT_h[:, :].rearrange("(kc p) n -> p kc n", p=128)

            # early DMAs, ordered so tile-0 chains can start ASAP
            def dma_x(t):
                tsl = slice(t * TOK_T, (t + 1) * TOK_T)
                nc.sync.dma_start(out=xsb[:, 0:4, tsl], in_=xin[:, 0:4, tsl])
                nc.sync.dma_start(out=xsb[:, 4:8, tsl], in_=xin[:, 4:8, tsl])

            nc.sync.dma_start(out=bqSb, in_=bq_h[:, :])
            nc.sync.dma_start(out=qksSb, in_=qks_h[:, :])
            nc.sync.dma_start(out=bvSb, in_=bv_h[:, :])
            nc.sync.dma_start(out=xsb[:, 0:4, 0:TOK_T], in_=xin[:, 0:4, 0:TOK_T])
            nc.sync.dma_start(out=Wsb[:, 1, :, :], in_=Wq_h[:, 1, :, :])
            nc.sync.dma_start(out=xsb[:, 4:8, 0:TOK_T], in_=xin[:, 4:8, 0:TOK_T])
            nc.sync.dma_start(out=Wsb[:, 0, :, :], in_=Wq_h[:, 0, :, :])
            dma_x(1)
            nc.sync.dma_start(out=Wsb[:, 2, :, :], in_=Wq_h[:, 2, :, :])
            for t in range(2, 4):
                dma_x(t)
            nc.sync.dma_start(out=WoSb, in_=Wo_h[:, :])
            for t in range(4, 8):
                dma_x(t)
            # PE p-state warm-up: harmless matmuls on a zeroed tile keep the
            # tensor engine ramping to full clock while the first DMAs land
            warm = consts.tile([128, TOK_T], bf16, tag="warm")
            nc.vector.memset(warm, 0.0)
            for w in range(12):
                pw = ps_o.tile([KB, TOK_T], f32, tag="po0", name="pw")
                nc.tensor.matmul(pw, warm[:, 0:128], warm,
                                 start=True, stop=True)
            make_identity(nc, ident)
            nc.vector.memset(onesrow, 1.0)
            nc.vector.memset(eps128, EPS)
            nc.vector.memset(zb128, 0.0)
            nc.vector.memset(ones2, 0.0)
            nc.vector.memset(ones2[0:64, 0:1], 1.0)
            nc.vector.memset(ones2[64:128, 1:2], 1.0)
            # vN ones + pad columns (softmax denominator trick)
            nc.vector.memset(vN[:, :, HD:HD + 1], 1.0)
            nc.vector.memset(vN[:, :, HD + 1:HD + 2], 0.0)
            nc.vector.memset(vN[:, :, VW + HD:VW + HD + 1], 1.0)
            nc.vector.memset(vN[:, :, VW + HD + 1:VW + HD + 2], 0.0)

            # ---------------- phase-1 pieces ----------------
            tile_state = {}

            def p_qkproj(t, m):
                # q (m=0) or k (m=1) projection; cast+bias; square
                tsl = slice(t * TOK_T, (t + 1) * TOK_T)
                px = ps_x.tile([128, TOK_T], f32, tag="px", name="pxqk")
                for kc in range(n_kc):
                    nc.tensor.matmul(
                        px,
                        Wsb[:, m, kc, :],
                        xsb[:, kc, tsl],
                        start=(kc == 0),
                        stop=(kc == n_kc - 1),
                    )
                raw = p1s.tile([128, TOK_T], bf16, tag=f"raw{m}",
                               name=f"raw{m}")
                nc.vector.tensor_scalar_add(raw, px, bqSb[:, m:m + 1])
                st = tile_state.setdefault(t, {})
                if "sq" not in st:
                    st["sq"] = p1s.tile([128, 2, TOK_T], bf16, tag="sq",
                                        name="sq", bufs=2)
                nc.vector.tensor_mul(st["sq"][:, m, :], raw, raw)
                st[f"raw{m}"] = raw

            def p_norm(t):
                # transposed mean-square stats + rstd = exp(-0.5 ln(ms+eps))
                st = tile_state[t]
                sq = st["sq"]
                msx = ps_x.tile([128, TOK_T], f32, tag="px", name="msx")
                msv = msx[:, 0:16].rearrange("p (c m h) -> p c m h", c=4, m=2)
                for c in range(4):
                    for m in range(2):
                        nc.tensor.matmul(
                            msv[:, c, m, :],
                            sq[:, m, c * 128:(c + 1) * 128],
                            ones2,
                            start=True, stop=True)
                lnv = p1s.tile([128, 16], f32, tag="lnv", name="lnv")
                nc.scalar.activation(out=lnv, in_=msx[:, 0:16], func=AF.Ln,
                                     bias=eps128[:, :], scale=1.0 / HD)
                rstdT = p1s.tile([128, 4, 2, 2], bf16, tag="rstdT",
                                 name="rstdT")
                nc.scalar.activation(
                    out=rstdT.rearrange("p c m h -> p (c m h)"), in_=lnv,
                    func=AF.Exp, bias=zb128[:, :], scale=-0.5)
                st["rstdT"] = rstdT

            def p_qkout(t, m):
                # broadcast rstd over head dims: materialize [tok, hd] rows
                # on the DVE (stride-0 read), then transpose via identity
                # matmul; normalize on the DVE
                st = tile_state[t]
                tsl = slice(t * TOK_T, (t + 1) * TOK_T)
                bcT = p1s.tile([128, 4, 128], bf16, tag="bcT", name="bcT")
                nc.vector.tensor_copy(
                    bcT.rearrange("p c (h x) -> p c h x", h=2),
                    st["rstdT"][:, :, m, :].unsqueeze(3).broadcast_to(
                        [128, 4, 2, HD]))
                bc = ps_x.tile([128, TOK_T], f32, tag="px", name="bc")
                for c in range(4):
                    nc.tensor.matmul(bc[:, c * 128:(c + 1) * 128],
                                     bcT[:, c, :], ident,
                                     start=True, stop=True)
                if m == 0:
                    nc.vector.scalar_tensor_tensor(
                        out=qnT[:, tsl], in0=st["raw0"],
                        scalar=qksSb[:, 0:1], in1=bc,
                        op0=ALU.mult, op1=ALU.mult)
                else:
                    nc.vector.tensor_mul(knT[:, tsl], st["raw1"], bc)

            def p_v(t):
                # v in natural [token, hd] layout (x stationary)
                pv = ps_x.tile([128, 4, 128], f32, tag="px", name="pv")
                for j in range(4):
                    tok0 = t * TOK_T + j * 128
                    for kc in range(n_kc):
                        nc.tensor.matmul(
                            pv[:, j, :],
                            xsb[:, kc, tok0:tok0 + 128],
                            Wsb[:, 2, kc, :],
                            start=(kc == 0),
                            stop=(kc == n_kc - 1) and not has_qkv_bias,
                        )
                    if has_qkv_bias:
                        nc.tensor.matmul(pv[:, j, :], onesrow[:, 0:128], bvSb,
                                         start=False, stop=True)
                c0 = t * 4
                dst = vN[:, c0:c0 + 4, :].rearrange(
                    "p c (h x) -> p c h x", h=2)[:, :, :, 0:HD]
                src = pv.rearrange("p c (h x) -> p c h x", h=2)
                nc.vector.tensor_copy(dst, src)

            # ---------------- work queue ----------------
            work = []
            pending_labels = set()

            def push(cost, fn, label=None):
                if label is not None:
                    pending_labels.add(label)
                work.append((cost, fn, label))

            def _pop_one():
                c, fn, label = work.pop(0)
                fn()
                if label is not None:
                    pending_labels.discard(label)
                return c

            def feed(budget):
                spent = 0
                while work and spent < budget:
                    spent += _pop_one()

            def drain_through(label):
                # emit queued pieces until `label` has been emitted; no-op if
                # the label was never pushed or already done
                while label in pending_labels:
                    _pop_one()

            def push_chain(t):
                # order chosen so PE-independent work (v proj) separates the
                # chain's cross-engine dependencies (cast/sq -> msq -> rstd)
                push(1800, lambda: p_qkproj(t, 1))
                push(1800, lambda: p_qkproj(t, 0))
                push(700, lambda: p_norm(t))
                push(700, lambda: p_qkout(t, 1), ("kout", t))
                push(700, lambda: p_qkout(t, 0), ("qout", t))
                push(2000, lambda: p_v(t), ("v", t))

            def push_chain_pair(a, b):
                # interleave two tiles' chains: tile b's projections hide
                # tile a's cross-engine (DVE/ACT) normalization latency
                push(1800, lambda: p_qkproj(a, 1))
                push(1800, lambda: p_qkproj(a, 0))
                push(1800, lambda: p_qkproj(b, 1))
                push(700, lambda: p_norm(a))
                push(1800, lambda: p_qkproj(b, 0))
                push(700, lambda: p_qkout(a, 1), ("kout", a))
                push(700, lambda: p_norm(b))
                push(700, lambda: p_qkout(a, 0), ("qout", a))
                push(700, lambda: p_qkout(b, 1), ("kout", b))
                push(2000, lambda: p_v(a), ("v", a))
                push(700, lambda: p_qkout(b, 0), ("qout", b))
                push(2000, lambda: p_v(b), ("v", b))

            # ---------------- attention unit ----------------
            uid_ctr = [0]

            def emit_unit(b, qt, last=False, next_q=None, defer_ops=False):
                uid = uid_ctr[0]
                uid_ctr[0] += 1
                q0 = b * S + qt * TOK_T
                qsl = slice(q0, q0 + TOK_T)
                drain_through(("qout", b * 4 + qt))
                onT = onTp.tile([PD, TOK_T], bf16, tag="onT", name="onT")
                po = {}
                pov = {}
                for h in range(HPC):
                    po[h] = ps_o.tile([KB, TOK_T], f32, tag=f"po{h}",
                                      name=f"po{h}")
                    pov[h] = po[h][:, 0:4 * VW].rearrange(
                        "p (a b) -> p a b", a=4)

                def finish_head(h, is_last_head=False):
                    def norm():
                        rec = recp.tile([128, 4, 1], f32, tag="rec",
                                        name="rec")
                        nc.vector.reciprocal(rec, pov[h][:, :, HD:HD + 1])
                        if ("on2", uid) not in tile_state:
                            tile_state[("on2", uid)] = onp.tile(
                                [128, 4, HPC, HD], bf16, tag="on2",
                                name="on2")
                        o_n2 = tile_state[("on2", uid)]
                        nc.vector.tensor_mul(
                            o_n2[:, :, h, :], pov[h][:, :, 0:HD],
                            rec.broadcast_to([128, 4, HD]))

                    def transp():
                        # both heads adjacent in free dim -> one 128x128
                        # xbar-tile DMA transpose per query sub-block; the
                        # last unit uses PE transposes instead (lower
                        # latency -- nothing hides the DMA turnaround there)
                        o_n2 = tile_state.pop(("on2", uid))
                        if last:
                            tp = ps_x.tile([PD, 4, 128], bf16, tag="px",
                                           name="tp")
                            for qs in range(4):
                                nc.tensor.transpose(
                                    tp[:, qs, :],
                                    o_n2[:, qs, :, :].rearrange(
                                        "p a b -> p (a b)"),
                                    ident)
                            nc.vector.tensor_copy(
                                onT[:, :].rearrange("p (a c) -> p a c", a=4),
                                tp)
                        else:
                            for qs in range(4):
                                nc.sync.dma_start_transpose(
                                    out=onT[:, qs * 128:(qs + 1) * 128],
                                    in_=o_n2[:, qs, :, :])

                    push(500, norm, ("norm", uid, h))
                    if is_last_head:
                        push(300, transp)

                def pv_chunk(pts_h, h, qs):
                    # one full accumulation group (all 16 kb) per qs slice:
                    # only one pending PSUM group per bank is legal
                    for kb in range(n_kb):
                        g, j = kb // NG, kb % NG
                        nc.tensor.matmul(
                            pov[h][:, qs, 0:HD + 1],
                            pts_h[g][:, j, qs * 128:(qs + 1) * 128],
                            vN[:, b * n_kb + kb, h * VW:h * VW + HD + 1],
                            start=(kb == 0),
                            stop=(kb == n_kb - 1),
                        )

                h_order = (1, 0) if last else (0, 1)
                pts = {h: [] for h in h_order}
                prev_h = None
                for h in h_order:
                    for g in range(n_kb // NG):
                        drain_through(("kout", b * 4 + g // 2))
                        hsl = slice(h * HD, (h + 1) * HD)
                        pss = ps4.tile([KB, NG, TOK_T], f32, tag="pss",
                                       name="pss")
                        for j in range(NG):
                            kb = g * NG + j
                            k0 = b * S + kb * KB
                            nc.tensor.matmul(pss[:, j, :],
                                             knT[hsl, k0:k0 + KB],
                                             qnT[hsl, qsl],
                                             start=True, stop=True)
                        pt = ptp.tile([KB, NG, TOK_T], bf16, tag="pt",
                                      name="pt")
                        nc.scalar.activation(out=pt, in_=pss, func=AF.Exp,
                                             bias=zb128[:, :], scale=1.0)
                        pts[h].append(pt)
                        if prev_h is not None and g % 2 == 1:
                            if g == 1:
                                drain_through(("v", b * 4 + 3))
                                if uid > 0:
                                    drain_through(("norm", uid - 1, prev_h))
                            pv_chunk(pts[prev_h], prev_h, g // 2)
                            if g == n_kb // NG - 1:
                                finish_head(prev_h)
                        if prev_h is not None and g == 4 and next_q is not None:
                            drain_through(next_q)
                        if prev_h is not None and g == 6:
                            drain_through(("kout", (b * 4 + 4) % 8))
                        feed(1100)
                    prev_h = h
                if uid > 0:
                    drain_through(("norm", uid - 1, prev_h))
                for qs in range(4):
                    push(600, (lambda qs=qs: pv_chunk(pts[prev_h], prev_h,
                                                      qs)))
                finish_head(prev_h, is_last_head=True)

                ot_ref = {}
                op_pieces = []
                for tb in range(4):
                    for od in range(2):
                        def oph(tb=tb, od=od):
                            if od == 0:
                                ot_ref[tb] = otp.tile([128, D], bf16,
                                                      tag="ot", name="ot")
                            ot = ot_ref[tb]
                            px = ps_x.tile([128, TOK_T], f32, tag="px",
                                           name="pxop")
                            nc.tensor.matmul(
                                px,
                                onT[:, tb * 128:(tb + 1) * 128],
                                WoSb[:, od * TOK_T:(od + 1) * TOK_T],
                                start=True, stop=True)
                            if last and od == 1:
                                nc.scalar.copy(
                                    out=ot[:, od * TOK_T:(od + 1) * TOK_T],
                                    in_=px)
                            else:
                                nc.vector.tensor_copy(
                                    ot[:, od * TOK_T:(od + 1) * TOK_T], px)
                            if od == 1:
                                r0 = q0 + tb * 128
                                nc.sync.dma_start(
                                    out=out_h[r0:r0 + 128, :], in_=ot)
                        op_pieces.append(oph)
                if defer_ops:
                    return op_pieces
                for p in op_pieces:
                    push(600, p)

            # ---------------- emission schedule ----------------
            # head: interleaved chains for tiles 0-1 (v deferred to feeds)
            p_qkproj(0, 1)
            p_qkproj(0, 0)
            p_qkproj(1, 1)
            p_norm(0)
            p_qkproj(1, 0)
            p_qkout(0, 1)
            p_norm(1)
            p_qkout(0, 0)
            p_qkout(1, 1)
            p_qkout(1, 0)
            push(2000, lambda: p_v(0), ("v", 0))
            push(2000, lambda: p_v(1), ("v", 1))
            push_chain_pair(2, 3)
            dops = []
            dops.append(emit_unit(0, 0, next_q=("qout", 1), defer_ops=True))
            push_chain_pair(4, 5)
            dops.append(emit_unit(0, 1, next_q=("qout", 2), defer_ops=True))
            dops.append(emit_unit(0, 2, next_q=("qout", 3), defer_ops=True))
            push_chain_pair(6, 7)
            dops.append(emit_unit(0, 3, next_q=("qout", 4), defer_ops=True))
            for p in dops[0]:
                push(600, p)
            emit_unit(1, 0, next_q=("qout", 5))
            for p in dops[1]:
                push(600, p)
            emit_unit(1, 1, next_q=("qout", 6))
            for p in dops[2]:
                push(600, p)
            emit_unit(1, 2, next_q=("qout", 7))
            for p in dops[3]:
                push(600, p)
            emit_unit(1, 3, last=True)
            feed(10 ** 9)  # drain

    nc.compile()
    return nc


def kernel(x, Wqkv, bqkv, Wout, bout, q_scale, k_scale):
    global _last_results
    import ml_dtypes
    from concourse.bass_utils import run_bass_kernel_spmd

    bf16 = ml_dtypes.bfloat16
    x = np.asarray(x, dtype=np.float32)
    Wqkv = np.asarray(Wqkv, dtype=np.float32)
    bqkv = np.asarray(bqkv, dtype=np.float32)
    Wout = np.asarray(Wout, dtype=np.float32)
    bout = np.asarray(bout, dtype=np.float32)
    q_scale = np.asarray(q_scale, dtype=np.float32)
    k_scale = np.asarray(k_scale, dtype=np.float32)

    has_qkv_bias = bool(np.any(bqkv != 0.0))

    xT = np.ascontiguousarray(x.reshape(N, D).T).astype(bf16)
    # k_scale folds into the q side (scores are bilinear in q, k)
    qks = (np.tile(q_scale * k_scale, HPC).reshape(PD, 1)
           / np.sqrt(HD)).astype(np.float32)

    in_maps = []
    for c in range(NCORES):
        c0 = c * PD
        # [128, m, kc, 128]: each m-slice DMA reads 2048B-contiguous rows
        Wq_s = np.ascontiguousarray(np.stack(
            [Wqkv[:, c0:c0 + PD], Wqkv[:, D + c0:D + c0 + PD],
             Wqkv[:, 2 * D + c0:2 * D + c0 + PD]],
            axis=1).reshape(8, 128, 3, PD).transpose(1, 2, 0, 3)).astype(bf16)
        bq_s = np.ascontiguousarray(np.stack(
            [bqkv[c0:c0 + PD], bqkv[D + c0:D + c0 + PD],
             bqkv[2 * D + c0:2 * D + c0 + PD]], axis=1))
        bv_s = bqkv[2 * D + c0:2 * D + c0 + PD].reshape(1, PD).astype(bf16)
        Wo_s = np.ascontiguousarray(Wout[c0:c0 + PD, :]).astype(bf16)
        in_maps.append({"xT": xT, "Wq": Wq_s, "bq": bq_s, "bv": bv_s,
                        "Wo": Wo_s, "qks": qks})

    nc = _build_program(has_qkv_bias)
    res = run_bass_kernel_spmd(nc, in_maps, core_ids=list(range(NCORES)))
    _last_results = res

    acc = res.results[0]["outp"].astype(np.float32)
    for c in range(1, NCORES):
        acc = acc + res.results[c]["outp"].astype(np.float32)
    acc = acc + bout
    return acc.reshape(B, S, D).astype(np.float32)
